# revision 1
# baseline (speedup 1.0000x reference)
"""KMLayer (Kuramoto oscillator layer) on 8 Trainium2 NeuronCores via Bass/Tile.

Strategy (row-sharded, output-node parallel):
  - A = sc[0] * conn_w  [N,N] is row-sharded: core r owns rows m in
    [r*M_LOC, (r+1)*M_LOC).  The shard is built once on-device (elementwise
    product of streamed sc/conn_w slabs), transposed through the PE array,
    and kept RESIDENT in SBUF as bf16 A^T [n-partition, m-free] (16 MB/core).
  - Each Euler step: coup.T = X^T-stationary matmul over the SBUF-resident
    A^T shard (4-way PE column tiling), a small fold-matmul transposes
    [bc, m] -> [m, bc] while summing the 4 column-tile partials, then the
    per-row update (tangent projection, omega rotation, pair renormalize)
    runs on DVE/ACT for the local rows only.
  - The new local state slab (cast to bf16) is AllGather'd across the 8
    cores each step so every core has the full X for the next matmul.
State is carried in fp32; only the matmul operands (A, gathered X) are bf16.
"""

import numpy as np
import ml_dtypes

import concourse.bass as bass
import concourse.mybir as mybir
import concourse.tile as tile
from concourse import bacc
from concourse.bass_utils import run_bass_kernel_spmd
from concourse.bass_interp import get_hw_module

F32 = mybir.dt.float32
BF16 = mybir.dt.bfloat16
ALU = mybir.AluOpType
ACTF = mybir.ActivationFunctionType
AXX = mybir.AxisListType.X

N_CORES = 8
B, C, N_FULL = 2, 16, 8192
BC = B * C  # 32
Q_STEPS = 8
GN_EPS = 1e-5
NRM_EPS = 1e-6


def _bcast(ap, parts):
    """Partition-broadcast view of a [1, f] DRAM AP -> [parts, f]."""
    return bass.AP(tensor=ap.tensor, offset=ap.offset, ap=[[0, parts]] + list(ap.ap[1:]))


def build_program(n=N_FULL, ncores=N_CORES, q_steps=Q_STEPS):
    m_loc = n // ncores            # rows owned per core
    mch = m_loc // 128             # 128-row chunks per core
    nch = n // 128                 # 128-col contraction chunks
    rg = [list(range(ncores))]

    nc = bacc.Bacc("TRN2", target_bir_lowering=False, debug=False,
                   enable_asserts=False, num_devices=ncores)

    # ---- I/O ----
    sc_s = nc.dram_tensor("sc_s", [m_loc, n], F32, kind="ExternalInput").ap()
    cw_s = nc.dram_tensor("cw_s", [m_loc, n], F32, kind="ExternalInput").ap()
    x_nat = nc.dram_tensor("x_nat", [BC, n], F32, kind="ExternalInput").ap()
    c_nat = nc.dram_tensor("c_nat", [BC, n], F32, kind="ExternalInput").ap()
    x_slab = nc.dram_tensor("x_slab", [BC, m_loc], F32, kind="ExternalInput").ap()
    c_slab = nc.dram_tensor("c_slab", [BC, m_loc], F32, kind="ExternalInput").ap()
    gnw_i = nc.dram_tensor("gnw_i", [BC, 1], F32, kind="ExternalInput").ap()
    gnb_i = nc.dram_tensor("gnb_i", [BC, 1], F32, kind="ExternalInput").ap()
    omg_i = nc.dram_tensor("omg_i", [1, mch * BC], F32, kind="ExternalInput").ap()
    gam_i = nc.dram_tensor("gam_i", [1, 1], F32, kind="ExternalInput").ap()
    sel2_i = nc.dram_tensor("sel2_i", [128, BC], F32, kind="ExternalInput").ap()
    id32_i = nc.dram_tensor("id32_i", [32, 32], F32, kind="ExternalInput").ap()
    id128_i = nc.dram_tensor("id128_i", [128, 128], BF16, kind="ExternalInput").ap()
    out_loc = nc.dram_tensor("out_loc", [q_steps, B, m_loc, C], F32,
                             kind="ExternalOutput").ap()

    with tile.TileContext(nc) as tc:
        with tc.tile_pool(name="consts", bufs=1) as consts, \
             tc.tile_pool(name="atbp", bufs=1) as atbp, \
             tc.tile_pool(name="state", bufs=2) as state, \
             tc.tile_pool(name="agd", bufs=2, space="DRAM") as agd:

            # ---------------- constants ----------------
            sel2_sb = consts.tile([128, BC], F32)
            nc.sync.dma_start(out=sel2_sb, in_=sel2_i)
            id32_sb = consts.tile([32, 32], F32)
            nc.sync.dma_start(out=id32_sb, in_=id32_i)
            id128_sb = consts.tile([128, 128], BF16)
            nc.sync.dma_start(out=id128_sb, in_=id128_i)
            gnw_sb = consts.tile([BC, 1], F32)
            nc.sync.dma_start(out=gnw_sb, in_=gnw_i)
            gnb_sb = consts.tile([BC, 1], F32)
            nc.sync.dma_start(out=gnb_sb, in_=gnb_i)
            omg_sb = consts.tile([128, mch * BC], F32)
            nc.sync.dma_start(out=omg_sb, in_=_bcast(omg_i, 128))
            gam_sb = consts.tile([128, 1], F32)
            nc.sync.dma_start(out=gam_sb, in_=_bcast(gam_i, 128))
            eps5_sb = consts.tile([BC, 1], F32)
            nc.vector.memset(eps5_sb, GN_EPS)
            eps6_sb = consts.tile([128, 1], F32)
            nc.vector.memset(eps6_sb, NRM_EPS)

            # persistent A^T shard [n_lo=128 part, (n_hi)(m_loc) free] bf16
            atb = atbp.tile([128, nch * m_loc], BF16)
            atb_r = atb.rearrange("p (t m) -> p t m", m=m_loc)

            # state tiles (tags shared with per-step allocations)
            xloc = state.tile([128, mch * BC], F32, tag="xloc")
            xcur = state.tile([128, nch * BC], BF16, tag="xcur")
            y_loc = consts.tile([128, mch * BC], F32)

            # ---------------- init: groupnorm stats + y + x0 ----------------
            with tc.tile_pool(name="initp", bufs=1) as initp, \
                 tc.tile_pool(name="psinit", bufs=2, space="PSUM") as psinit:
                # -- groupnorm statistics over full c --
                c128 = initp.tile([128, n // 4], F32, tag="ibig")
                nc.sync.dma_start(out=c128,
                                  in_=c_nat.rearrange("a (q m) -> (a q) m", q=4))
                fsub = n // 4
                nsub = 1
                while fsub > 512:
                    assert fsub % 2 == 0
                    fsub //= 2
                    nsub *= 2
                stats = initp.tile([128, nsub, 6], F32)
                c128v = c128.rearrange("p (s m) -> p s m", s=nsub)
                for s in range(nsub):
                    nc.vector.bn_stats(out=stats[:, s, :], in_=c128v[:, s, :])
                mv = initp.tile([128, 2], F32)
                nc.vector.bn_aggr(out=mv, in_=stats)
                # mv[:,1] <- E[x^2] = mean^2 + var
                nc.vector.scalar_tensor_tensor(
                    out=mv[:, 1:2], in0=mv[:, 0:1], scalar=mv[:, 0:1],
                    in1=mv[:, 1:2], op0=ALU.mult, op1=ALU.add)
                ps_s = psinit.tile([32, 2], F32, tag="ps_small")
                nc.tensor.matmul(ps_s, lhsT=sel2_sb, rhs=mv, start=True, stop=True)
                mvg = initp.tile([BC, 2], F32)
                nc.vector.tensor_copy(mvg, ps_s)
                mu2 = initp.tile([BC, 1], F32)
                nc.vector.tensor_mul(mu2, mvg[:, 0:1], mvg[:, 0:1])
                var32 = initp.tile([BC, 1], F32)
                nc.vector.tensor_sub(var32, mvg[:, 1:2], mu2)
                sd32 = initp.tile([BC, 1], F32)
                nc.scalar.activation(out=sd32, in_=var32, func=ACTF.Sqrt,
                                     bias=eps5_sb, scale=1.0)
                rstd = initp.tile([BC, 1], F32)
                nc.vector.reciprocal(out=rstd, in_=sd32)
                scl32 = initp.tile([BC, 1], F32)
                nc.vector.tensor_mul(scl32, rstd, gnw_sb)
                nmu = initp.tile([BC, 1], F32)
                nc.vector.tensor_scalar_mul(nmu, mvg[:, 0:1], -1.0)
                bia32 = initp.tile([BC, 1], F32)
                nc.vector.scalar_tensor_tensor(
                    out=bia32, in0=nmu, scalar=scl32, in1=gnb_sb,
                    op0=ALU.mult, op1=ALU.add)

                # -- y (normalized c) for the local slab, transposed --
                csl = initp.tile([BC, m_loc], F32)
                nc.sync.dma_start(out=csl, in_=c_slab)
                ysl = initp.tile([BC, m_loc], F32)
                nc.scalar.activation(out=ysl, in_=csl, func=ACTF.Identity,
                                     bias=bia32, scale=scl32)
                ps_y = psinit.tile([128, mch * BC], F32, tag="ps_y")
                for mc in range(mch):
                    nc.tensor.transpose(ps_y[:, mc * BC:(mc + 1) * BC],
                                        ysl[:, mc * 128:(mc + 1) * 128], id32_sb)
                nc.vector.tensor_copy(y_loc, ps_y)

                def pair_normalize(src, npairs, dst_a, dst_b, pool):
                    """dst = src / (||pair||+eps); writes dst_a (f32 or None)
                    and dst_b (second tile or None), given src [128, 2*npairs]."""
                    sq = pool.tile([128, 2 * npairs], F32, tag="pn_sq")
                    nc.vector.tensor_mul(sq, src, src)
                    ss = pool.tile([128, npairs], F32, tag="pn_ss")
                    nc.vector.tensor_reduce(
                        ss, sq.rearrange("p (g two) -> p g two", two=2),
                        axis=AXX, op=ALU.add)
                    nr = pool.tile([128, npairs], F32, tag="pn_nr")
                    nc.scalar.activation(out=nr, in_=ss, func=ACTF.Sqrt)
                    nc.scalar.activation(out=nr, in_=nr, func=ACTF.Identity,
                                         bias=eps6_sb)
                    rr = pool.tile([128, npairs], F32, tag="pn_rr")
                    nc.vector.reciprocal_approx_fast(out=rr, in_=nr)
                    sv = src.rearrange("p (g two) -> p g two", two=2)
                    for dst in (dst_a, dst_b):
                        if dst is None:
                            continue
                        dv = dst.rearrange("p (g two) -> p g two", two=2)
                        nc.vector.tensor_mul(dv[:, :, 0], sv[:, :, 0], rr)
                        nc.vector.tensor_mul(dv[:, :, 1], sv[:, :, 1], rr)

                # -- x0: full transposed state + pair-normalize (fp32 -> bf16) --
                nhalf = 2 if n >= 4096 else 1
                nch_h = nch // nhalf
                x0f = initp.tile([128, nch_h * BC], F32)
                tpg = min(16, nch_h)  # transposes per psum tile
                for hh in range(nhalf):
                    xf = initp.tile([BC, n // nhalf], F32, tag="ibig")
                    nc.sync.dma_start(
                        out=xf, in_=x_nat[:, hh * (n // nhalf):(hh + 1) * (n // nhalf)])
                    for tg in range(nch_h // tpg):
                        ps_x = psinit.tile([128, tpg * BC], F32, tag="ps_x")
                        for tt in range(tpg):
                            t = tg * tpg + tt
                            nc.tensor.transpose(ps_x[:, tt * BC:(tt + 1) * BC],
                                                xf[:, t * 128:(t + 1) * 128], id32_sb)
                        nc.vector.tensor_copy(
                            x0f[:, tg * tpg * BC:(tg + 1) * tpg * BC], ps_x)
                    pair_normalize(x0f, nch_h * BC // 2,
                                   xcur[:, hh * nch_h * BC:(hh + 1) * nch_h * BC],
                                   None, initp)

                # local x0 (fp32) from the per-core slab input
                xsl = initp.tile([BC, m_loc], F32)
                nc.sync.dma_start(out=xsl, in_=x_slab)
                xl_pre = initp.tile([128, mch * BC], F32)
                ps_xl = psinit.tile([128, mch * BC], F32, tag="ps_y")
                for mc in range(mch):
                    nc.tensor.transpose(ps_xl[:, mc * BC:(mc + 1) * BC],
                                        xsl[:, mc * 128:(mc + 1) * 128], id32_sb)
                nc.vector.tensor_copy(xl_pre, ps_xl)
                pair_normalize(xl_pre, mch * BC // 2, xloc, None, initp)

            # ---------------- build A^T shard ----------------
            piece = min(1024, n)
            with tc.tile_pool(name="bstage", bufs=2) as bstage, \
                 tc.tile_pool(name="bprod", bufs=1) as bprod, \
                 tc.tile_pool(name="pst", bufs=2, space="PSUM") as pst:
                for j0 in range(0, mch, 2):
                    nh = min(2, mch - j0)
                    prods = []
                    for h in range(nh):
                        pr = bprod.tile([128, n], BF16, tag=f"prod{h}")
                        prods.append(pr)
                        for qq in range(n // piece):
                            scp = bstage.tile([128, piece], F32, tag="scp")
                            nc.sync.dma_start(
                                out=scp,
                                in_=sc_s[(j0 + h) * 128:(j0 + h + 1) * 128,
                                         qq * piece:(qq + 1) * piece])
                            cwp = bstage.tile([128, piece], F32, tag="cwp")
                            nc.sync.dma_start(
                                out=cwp,
                                in_=cw_s[(j0 + h) * 128:(j0 + h + 1) * 128,
                                         qq * piece:(qq + 1) * piece])
                            nc.vector.tensor_mul(
                                pr[:, qq * piece:(qq + 1) * piece], scp, cwp)
                    tpg2 = min(8, nch)
                    for tg in range(nch // tpg2):
                        pt = pst.tile([128, tpg2 * nh * 128], BF16)
                        for tt in range(tpg2):
                            t = tg * tpg2 + tt
                            for h in range(nh):
                                nc.tensor.transpose(
                                    pt[:, (tt * nh + h) * 128:(tt * nh + h + 1) * 128],
                                    prods[h][:, t * 128:(t + 1) * 128], id128_sb)
                        src = pt.rearrange("p (t h k) -> p t h k", t=tpg2, h=nh)
                        dst = atb.rearrange("p (t j k) -> p t j k",
                                            t=nch, j=mch)[:, tg * tpg2:(tg + 1) * tpg2,
                                                          j0:j0 + nh, :]
                        nc.scalar.copy(out=dst, in_=src)

            # ---------------- Euler steps ----------------
            steps_ctx = tc.tile_pool(name="psmm", bufs=1, space="PSUM")
            psf_ctx = tc.tile_pool(name="psf", bufs=2, space="PSUM")
            ew_ctx = tc.tile_pool(name="ew", bufs=2)
            with steps_ctx as psmm, psf_ctx as psf, ew_ctx as ew:
              mq = m_loc // 4  # m-range per PE column-tile group
              for k in range(q_steps):
                  # each col-tile group j accumulates its own m-quarter in its
                  # own PSUM bank (bank stride 512 fp32 = 2 KiB)
                  psa = psmm.tile([128, 4, 512], F32)
                  for ncnk in range(nch):
                      for j in range(4):
                          nc.tensor.matmul(
                              psa[32 * j:32 * (j + 1), j, 0:mq],
                              lhsT=xcur[:, ncnk * BC:(ncnk + 1) * BC],
                              rhs=atb_r[:, ncnk, j * mq:(j + 1) * mq],
                              start=(ncnk == 0), stop=(ncnk == nch - 1),
                              tile_position=(0, 32 * j))
                  # cross-quadrant DVE evictions -> coup.T [32 bc, m_loc]
                  coupT = ew.tile([32, m_loc], F32, tag="coupT")
                  for j in range(4):
                      nc.vector.tensor_copy(coupT[:, j * mq:(j + 1) * mq],
                                            psa[32 * j:32 * (j + 1), j, 0:mq])
                  # PE transposes -> coup [m partitions, bc]
                  psb = psf.tile([128, mch * BC], F32)
                  for mc in range(mch):
                      nc.tensor.transpose(psb[:, mc * BC:(mc + 1) * BC],
                                          coupT[:, mc * 128:(mc + 1) * 128],
                                          id32_sb)
                  # elementwise update on [128, mch*BC]
                  fw = mch * BC
                  yt = ew.tile([128, fw], F32, tag="yt")
                  nc.vector.scalar_tensor_tensor(out=yt, in0=psb, scalar=1.0,
                                                 in1=y_loc, op0=ALU.mult, op1=ALU.add)
                  pr_t = ew.tile([128, fw], F32, tag="pr_t")
                  nc.vector.tensor_mul(pr_t, xloc, yt)
                  sim = ew.tile([128, fw // 2], F32, tag="sim")
                  nc.vector.tensor_reduce(
                      sim, pr_t.rearrange("p (g two) -> p g two", two=2),
                      axis=AXX, op=ALU.add)
                  xl3 = xloc.rearrange("p (g two) -> p g two", two=2)
                  yt3 = yt.rearrange("p (g two) -> p g two", two=2)
                  tmp = ew.tile([128, fw], F32, tag="tmp")
                  tm3 = tmp.rearrange("p (g two) -> p g two", two=2)
                  proj = ew.tile([128, fw], F32, tag="proj")
                  pj3 = proj.rearrange("p (g two) -> p g two", two=2)
                  nc.vector.tensor_mul(tm3[:, :, 0], sim, xl3[:, :, 0])
                  nc.vector.tensor_mul(tm3[:, :, 1], sim, xl3[:, :, 1])
                  nc.vector.tensor_sub(proj, yt, tmp)
                  omg3 = omg_sb.rearrange("p (g two) -> p g two", two=2)
                  nc.vector.tensor_mul(tm3[:, :, 0], xl3[:, :, 1], omg3[:, :, 0])
                  nc.vector.tensor_mul(tm3[:, :, 1], xl3[:, :, 0], omg3[:, :, 1])
                  tsum = ew.tile([128, fw], F32, tag="tsum")
                  nc.vector.tensor_add(tsum, proj, tmp)
                  xn_pre = ew.tile([128, fw], F32, tag="xn_pre")
                  nc.vector.scalar_tensor_tensor(out=xn_pre, in0=tsum, scalar=gam_sb,
                                                 in1=xloc, op0=ALU.mult, op1=ALU.add)
                  xn = state.tile([128, fw], F32, tag="xloc")
                  pair_normalize(xn_pre, fw // 2, xn, None, ew)
                  # stream the step's state slab out
                  xn4 = xn.rearrange("p (mh b c) -> p mh b c", b=B, c=C)
                  for bb in range(B):
                      nc.sync.dma_start(
                          out=out_loc[k, bb].rearrange("(mh p) c -> p mh c", p=128),
                          in_=xn4[:, :, bb, :])
                  xloc = xn
                  if k < q_steps - 1:
                      xbf = ew.tile([128, fw], BF16, tag="xbf")
                      nc.scalar.copy(out=xbf, in_=xn)
                      agi = agd.tile([m_loc, BC], BF16, tag="agi")
                      nc.sync.dma_start(
                          out=agi.rearrange("(mh p) c -> p mh c", p=128),
                          in_=xbf.rearrange("p (mh c) -> p mh c", c=BC))
                      ago = agd.tile([n, BC], BF16, tag="ago")
                      nc.gpsimd.collective_compute(
                          "AllGather", ALU.bypass, replica_groups=rg,
                          ins=[agi.opt()], outs=[ago.opt()])
                      xnew = state.tile([128, nch * BC], BF16, tag="xcur")
                      nc.sync.dma_start(
                          out=xnew.rearrange("p (t c) -> p t c", c=BC),
                          in_=ago.rearrange("(t p) c -> p t c", p=128))
                      xcur = xnew

    nc.compile()
    nc.m = get_hw_module(nc.m)
    return nc


def make_inputs(x, c, sc, gn_w, gn_b, conn_w, omg_param, gamma,
                n=N_FULL, ncores=N_CORES):
    """Host-side marshalling: per-core input dicts."""
    m_loc = n // ncores
    mch = m_loc // 128
    bf16 = ml_dtypes.bfloat16

    x_nat = np.ascontiguousarray(x.reshape(BC, n), dtype=np.float32)
    c_nat = np.ascontiguousarray(c.reshape(BC, n), dtype=np.float32)
    gnw_i = np.ascontiguousarray(np.tile(gn_w.astype(np.float32), B)[:, None])
    gnb_i = np.ascontiguousarray(np.tile(gn_b.astype(np.float32), B)[:, None])

    omg = np.abs(omg_param.astype(np.float32)[:, 0])  # [C//2]
    row = np.empty(BC, np.float32)
    for b in range(B):
        for g in range(C // 2):
            row[b * C + 2 * g] = omg[g]
            row[b * C + 2 * g + 1] = -omg[g]
    omg_i = np.ascontiguousarray(np.tile(row, mch)[None, :])

    gam_i = np.asarray(gamma, np.float32).reshape(1, 1)

    sel2 = np.zeros((128, BC), np.float32)
    for p in range(128):
        for j in range(BC):
            if (p // 4) // 2 == j // 2:
                sel2[p, j] = 1.0 / 8.0
    id32 = np.eye(32, dtype=np.float32)
    id128 = np.eye(128).astype(bf16)

    shared = dict(x_nat=x_nat, c_nat=c_nat, gnw_i=gnw_i, gnb_i=gnb_i,
                  omg_i=omg_i, gam_i=gam_i, sel2_i=sel2,
                  id32_i=id32, id128_i=id128)
    in_maps = []
    for r in range(ncores):
        sl = slice(r * m_loc, (r + 1) * m_loc)
        in_maps.append(dict(
            shared,
            sc_s=np.ascontiguousarray(sc[0, sl, :], dtype=np.float32),
            cw_s=np.ascontiguousarray(conn_w[sl, :], dtype=np.float32),
            x_slab=np.ascontiguousarray(x_nat[:, sl]),
            c_slab=np.ascontiguousarray(c_nat[:, sl]),
        ))
    return in_maps


_PROGRAM_CACHE = {}


def get_program(n=N_FULL, ncores=N_CORES, q_steps=Q_STEPS):
    key = (n, ncores, q_steps)
    if key not in _PROGRAM_CACHE:
        _PROGRAM_CACHE[key] = build_program(n, ncores, q_steps)
    return _PROGRAM_CACHE[key]


def kernel(x, c, sc, gn_w, gn_b, conn_w, omg_param, gamma, Q):
    assert int(Q) == Q_STEPS
    x = np.asarray(x); c = np.asarray(c); sc = np.asarray(sc)
    gn_w = np.asarray(gn_w); gn_b = np.asarray(gn_b)
    conn_w = np.asarray(conn_w); omg_param = np.asarray(omg_param)
    gamma = np.asarray(gamma)
    n = x.shape[2]
    nc = get_program(n, N_CORES, Q_STEPS)
    in_maps = make_inputs(x, c, sc, gn_w, gn_b, conn_w, omg_param, gamma,
                          n=n, ncores=N_CORES)
    res = run_bass_kernel_spmd(nc, in_maps, core_ids=list(range(N_CORES)))
    outs = [res.results[r]["out_loc"] for r in range(N_CORES)]
    return np.ascontiguousarray(np.concatenate(outs, axis=2), dtype=np.float32)



# revision 5
# speedup vs baseline: 121.5649x; 121.5649x over previous
"""KMLayer (Kuramoto oscillator layer) on 8 Trainium2 NeuronCores via Bass/Tile.

Strategy (row-sharded, output-node parallel), v2:
  - A = sc[0] * conn_w  [N,N] row-sharded: core r owns rows m in
    [r*M_LOC, (r+1)*M_LOC).  Built on-device from 1MB streamed sc/cw slabs,
    transposed through the PE array, kept RESIDENT in SBUF as bf16 A^T
    [n-partition, m-free] (16 MB/core) in 4 column-group tiles so step-0
    matmuls overlap the build DMA.
  - Channel order is permuted host-side to (eo, b, g) ("even-first") so the
    per-step update runs on contiguous even/odd half tiles with no strided
    pair reductions.
  - Each Euler step: x-stationary matmuls over the resident A^T shard
    (2-way PE column tiling, N=512), PE transpose back to [m, bc], a lean
    DVE/ACT update chain, then the new local slab (bf16, p-major layout)
    is AllGather'd; the gather-in DMA reads 512-byte lines.
  - Outputs accumulate in SBUF; one 1MB DMA at the end; host reassembles.
State is carried in fp32; matmul operands (A, gathered X) are bf16.
"""

import numpy as np
import ml_dtypes

import concourse.bass as bass
import concourse.mybir as mybir
import concourse.tile as tile
from concourse import bacc
from concourse.bass_utils import run_bass_kernel_spmd
from concourse.bass_interp import get_hw_module

F32 = mybir.dt.float32
BF16 = mybir.dt.bfloat16
ALU = mybir.AluOpType
ACTF = mybir.ActivationFunctionType
AXX = mybir.AxisListType.X

N_CORES = 8
B, C, N_FULL = 2, 16, 8192
BC = B * C  # 32
HBC = BC // 2  # 16: even (or odd) half of the channel axis
Q_STEPS = 8
GN_EPS = 1e-5
NRM_EPS2 = 1e-12  # guards Rsqrt(ss); ref adds 1e-6 to the norm instead
N_GROUPS = 4      # A^T column-group tiles (build/step-0 overlap granularity)


def _bcast(ap, parts):
    """Partition-broadcast view of a [1, f] DRAM AP -> [parts, f]."""
    return bass.AP(tensor=ap.tensor, offset=ap.offset, ap=[[0, parts]] + list(ap.ap[1:]))


def build_program(n=N_FULL, ncores=N_CORES, q_steps=Q_STEPS):
    m_loc = n // ncores            # rows owned per core
    mch = m_loc // 128             # 128-row chunks per core (8)
    nch = n // 128                 # 128-col contraction chunks (64)
    gch = nch // N_GROUPS          # n-chunks per A^T group (16)
    mh2 = m_loc // 2               # m-range per PE column-tile group (512)
    rg = [list(range(ncores))]
    fw = mch * BC                  # per-node free width (256)
    hw = fw // 2                   # even/odd half width (128)

    nc = bacc.Bacc("TRN2", target_bir_lowering=False, debug=False,
                   enable_asserts=False, num_devices=ncores)

    # ---- I/O ----
    sc_s = nc.dram_tensor("sc_s", [m_loc, n], F32, kind="ExternalInput").ap()
    cw_s = nc.dram_tensor("cw_s", [m_loc, n], F32, kind="ExternalInput").ap()
    c_nat = nc.dram_tensor("c_nat", [BC, n], F32, kind="ExternalInput").ap()
    x_slab = nc.dram_tensor("x_slab", [BC, m_loc], F32, kind="ExternalInput").ap()
    c_slab = nc.dram_tensor("c_slab", [BC, m_loc], F32, kind="ExternalInput").ap()
    gnw_i = nc.dram_tensor("gnw_i", [BC, 1], F32, kind="ExternalInput").ap()
    gnb_i = nc.dram_tensor("gnb_i", [BC, 1], F32, kind="ExternalInput").ap()
    omg_i = nc.dram_tensor("omg_i", [1, 2 * hw], F32, kind="ExternalInput").ap()
    gam_i = nc.dram_tensor("gam_i", [1, 1], F32, kind="ExternalInput").ap()
    sel2_i = nc.dram_tensor("sel2_i", [128, BC], F32, kind="ExternalInput").ap()
    id32_i = nc.dram_tensor("id32_i", [32, 32], F32, kind="ExternalInput").ap()
    id128_i = nc.dram_tensor("id128_i", [128, 128], BF16, kind="ExternalInput").ap()
    # [p, k, mh, eo, b, g] f32 -- host reassembles
    out_d = nc.dram_tensor("out_d", [128, q_steps * fw], F32,
                           kind="ExternalOutput").ap()

    with tile.TileContext(nc) as tc:
        with tc.tile_pool(name="consts", bufs=1) as consts, \
             tc.tile_pool(name="atbp", bufs=1) as atbp, \
             tc.tile_pool(name="state", bufs=2) as state, \
             tc.tile_pool(name="outp", bufs=1) as outp, \
             tc.tile_pool(name="agd", bufs=2, space="DRAM") as agd, \
             tc.tile_pool(name="psmm", bufs=1, space="PSUM") as psmm:

            # ---------------- constants ----------------
            sel2_sb = consts.tile([128, BC], F32)
            nc.sync.dma_start(out=sel2_sb, in_=sel2_i)
            id32_sb = consts.tile([32, 32], F32)
            nc.sync.dma_start(out=id32_sb, in_=id32_i)
            id128_sb = consts.tile([128, 128], BF16)
            nc.sync.dma_start(out=id128_sb, in_=id128_i)
            gnw_sb = consts.tile([BC, 1], F32)
            nc.sync.dma_start(out=gnw_sb, in_=gnw_i)
            gnb_sb = consts.tile([BC, 1], F32)
            nc.sync.dma_start(out=gnb_sb, in_=gnb_i)
            omg_sb = consts.tile([128, 2 * hw], F32)   # [:, :hw]=+w, [:, hw:]=-w
            nc.sync.dma_start(out=omg_sb, in_=_bcast(omg_i, 128))
            gam_sb = consts.tile([128, 1], F32)
            nc.sync.dma_start(out=gam_sb, in_=_bcast(gam_i, 128))
            eps5_sb = consts.tile([BC, 1], F32)
            nc.vector.memset(eps5_sb, GN_EPS)
            eps12_sb = consts.tile([128, 1], F32)
            nc.vector.memset(eps12_sb, NRM_EPS2)
            y_e = consts.tile([128, hw], F32)
            y_o = consts.tile([128, hw], F32)

            # persistent A^T shard: 4 group tiles, each
            # [128 n_lo, (gch n_hi)(m_loc) free] bf16
            atbs = []
            for g in range(N_GROUPS):
                atb_g = atbp.tile([128, gch * m_loc], BF16, tag=f"atb{g}")
                atbs.append(atb_g)

            # state: local even/odd halves (f32) + gathered full x (bf16)
            xe = state.tile([128, hw], F32, tag="xe")
            xo = state.tile([128, hw], F32, tag="xo")
            xcur = state.tile([128, nch * BC], BF16, tag="xcur")

            # output accumulator [p, (k mh eo bg)] f32
            outacc = outp.tile([128, q_steps * fw], F32)
            oa_r = outacc.rearrange("p (k mh eo bg) -> p k mh eo bg",
                                    k=q_steps, mh=mch, eo=2)

            def exchange(sl_tile, xcur_dst):
                """slab [128, fw] bf16 -> AllGather -> xcur_dst [128, nch*BC]."""
                agi = agd.tile([128, fw], BF16, tag="agi")
                nc.sync.dma_start(out=agi, in_=sl_tile)
                ago = agd.tile([ncores * 128, fw], BF16, tag="ago")
                nc.gpsimd.collective_compute(
                    "AllGather", ALU.bypass, replica_groups=rg,
                    ins=[agi.opt()], outs=[ago.opt()])
                nc.sync.dma_start(
                    out=xcur_dst.rearrange("p (r f) -> p r f", r=ncores),
                    in_=ago.rearrange("(r p) f -> p r f", p=128))

            # ---------------- init: groupnorm stats + y + x0 ----------------
            with tc.tile_pool(name="initp", bufs=1) as initp, \
                 tc.tile_pool(name="psinit", bufs=2, space="PSUM") as psinit:
                # -- groupnorm statistics over full c (natural order) --
                c128 = initp.tile([128, n // 4], F32, tag="ibig")
                nc.sync.dma_start(out=c128,
                                  in_=c_nat.rearrange("a (q m) -> (a q) m", q=4))
                fsub = n // 4
                nsub = 1
                while fsub > 512:
                    assert fsub % 2 == 0
                    fsub //= 2
                    nsub *= 2
                stats = initp.tile([128, nsub, 6], F32)
                c128v = c128.rearrange("p (s m) -> p s m", s=nsub)
                for s in range(nsub):
                    nc.vector.bn_stats(out=stats[:, s, :], in_=c128v[:, s, :])
                mv = initp.tile([128, 2], F32)
                nc.vector.bn_aggr(out=mv, in_=stats)
                # mv[:,1] <- E[x^2] = mean^2 + var
                nc.vector.scalar_tensor_tensor(
                    out=mv[:, 1:2], in0=mv[:, 0:1], scalar=mv[:, 0:1],
                    in1=mv[:, 1:2], op0=ALU.mult, op1=ALU.add)
                ps_s = psinit.tile([32, 2], F32, tag="ps_small")
                nc.tensor.matmul(ps_s, lhsT=sel2_sb, rhs=mv, start=True, stop=True)
                mvg = initp.tile([BC, 2], F32)
                nc.vector.tensor_copy(mvg, ps_s)
                mu2 = initp.tile([BC, 1], F32)
                nc.vector.tensor_mul(mu2, mvg[:, 0:1], mvg[:, 0:1])
                var32 = initp.tile([BC, 1], F32)
                nc.vector.tensor_sub(var32, mvg[:, 1:2], mu2)
                sd32 = initp.tile([BC, 1], F32)
                nc.scalar.activation(out=sd32, in_=var32, func=ACTF.Sqrt,
                                     bias=eps5_sb, scale=1.0)
                rstd = initp.tile([BC, 1], F32)
                nc.vector.reciprocal(out=rstd, in_=sd32)
                scl32 = initp.tile([BC, 1], F32)
                nc.vector.tensor_mul(scl32, rstd, gnw_sb)
                nmu = initp.tile([BC, 1], F32)
                nc.vector.tensor_scalar_mul(nmu, mvg[:, 0:1], -1.0)
                bia32 = initp.tile([BC, 1], F32)
                nc.vector.scalar_tensor_tensor(
                    out=bia32, in0=nmu, scalar=scl32, in1=gnb_sb,
                    op0=ALU.mult, op1=ALU.add)

                # -- y (normalized c) for the local slab, transposed --
                # c_slab rows are already host-permuted to (eo, b, g) order
                csl = initp.tile([BC, m_loc], F32, tag="isl")
                nc.sync.dma_start(out=csl, in_=c_slab)
                ysl = initp.tile([BC, m_loc], F32, tag="isl2")
                nc.scalar.activation(out=ysl, in_=csl, func=ACTF.Identity,
                                     bias=bia32, scale=scl32)
                ps_y = psinit.tile([128, fw], F32, tag="ps_y")
                for mc in range(mch):
                    nc.tensor.transpose(ps_y[:, mc * BC:(mc + 1) * BC],
                                        ysl[:, mc * 128:(mc + 1) * 128], id32_sb)
                yfull = initp.tile([128, fw], F32)
                nc.vector.tensor_copy(yfull, ps_y)
                yv = yfull.rearrange("p (mh eo h) -> p mh eo h", eo=2, h=HBC)
                nc.vector.tensor_copy(
                    y_e.rearrange("p (mh h) -> p mh h", h=HBC), yv[:, :, 0])
                nc.vector.tensor_copy(
                    y_o.rearrange("p (mh h) -> p mh h", h=HBC), yv[:, :, 1])

                # -- x0 local slab: transpose + pair-normalize (even/odd) --
                xsl = initp.tile([BC, m_loc], F32, tag="isl")
                nc.sync.dma_start(out=xsl, in_=x_slab)
                ps_x = psinit.tile([128, fw], F32, tag="ps_y")
                for mc in range(mch):
                    nc.tensor.transpose(ps_x[:, mc * BC:(mc + 1) * BC],
                                        xsl[:, mc * 128:(mc + 1) * 128], id32_sb)
                x0f = initp.tile([128, fw], F32)
                nc.vector.tensor_copy(x0f, ps_x)
                x0v = x0f.rearrange("p (mh eo h) -> p mh eo h", eo=2, h=HBC)
                xev = xe.rearrange("p (mh h) -> p mh h", h=HBC)
                xov = xo.rearrange("p (mh h) -> p mh h", h=HBC)
                t2 = initp.tile([128, hw], F32, tag="t2")
                u3 = initp.tile([128, hw], F32, tag="u3")
                nc.scalar.activation(out=t2.rearrange("p (mh h) -> p mh h", h=HBC),
                                     in_=x0v[:, :, 0], func=ACTF.Square)
                nc.scalar.activation(out=u3.rearrange("p (mh h) -> p mh h", h=HBC),
                                     in_=x0v[:, :, 1], func=ACTF.Square)
                ss = initp.tile([128, hw], F32, tag="ss")
                nc.vector.tensor_add(ss, t2, u3)
                nr = initp.tile([128, hw], F32, tag="nr")
                nc.scalar.activation(out=nr, in_=ss, func=ACTF.Sqrt,
                                     bias=eps12_sb, scale=1.0)
                rr = initp.tile([128, hw], F32, tag="rr")
                nc.vector.reciprocal_approx_fast(out=rr, in_=nr)
                nc.vector.tensor_mul(xev, x0v[:, :, 0], rr.rearrange(
                    "p (mh h) -> p mh h", h=HBC))
                nc.vector.tensor_mul(xov, x0v[:, :, 1], rr.rearrange(
                    "p (mh h) -> p mh h", h=HBC))
                # slab for AG#0
                sl0 = state.tile([128, fw], BF16, tag="sl")
                sl0v = sl0.rearrange("p (mh eo h) -> p mh eo h", eo=2, h=HBC)
                nc.vector.tensor_copy(sl0v[:, :, 0], xev)
                nc.vector.tensor_copy(sl0v[:, :, 1], xov)
                exchange(sl0, xcur)

            # ---------------- build A^T shard (+ step-0 matmuls) -----------
            # psa: 2-way column tiling, group j -> psum partitions 64j..64j+31,
            # cols 0..511 (m-half j)
            psa = psmm.tile([128, mh2], F32)
            piece = 2048
            with tc.tile_pool(name="bstage", bufs=2) as bstage, \
                 tc.tile_pool(name="bprod", bufs=2) as bprod, \
                 tc.tile_pool(name="pst", bufs=2, space="PSUM") as pst:
                for g in range(N_GROUPS):
                    atb = atbs[g]
                    atb_r = atb.rearrange("p (t m) -> p t m", m=m_loc)
                    for j in range(mch):
                        scp = bstage.tile([128, piece], F32, tag="scp")
                        nc.sync.dma_start(
                            out=scp,
                            in_=sc_s[j * 128:(j + 1) * 128,
                                     g * piece:(g + 1) * piece])
                        cwp = bstage.tile([128, piece], F32, tag="cwp")
                        nc.sync.dma_start(
                            out=cwp,
                            in_=cw_s[j * 128:(j + 1) * 128,
                                     g * piece:(g + 1) * piece])
                        prod = bprod.tile([128, piece], BF16, tag="prod")
                        nc.vector.tensor_mul(prod, scp, cwp)
                        for half in range(2):
                            pt = pst.tile([128, 8 * 128], BF16, tag="pt")
                            for tt in range(8):
                                t = half * 8 + tt
                                nc.tensor.transpose(
                                    pt[:, tt * 128:(tt + 1) * 128],
                                    prod[:, t * 128:(t + 1) * 128], id128_sb)
                            dst = atb_r[:, half * 8:(half + 1) * 8,
                                        j * 128:(j + 1) * 128]
                            nc.scalar.copy(
                                out=dst,
                                in_=pt.rearrange("p (t m) -> p t m", m=128))
                    # step-0 matmuls for this group's n-chunks
                    for tl in range(gch):
                        t = g * gch + tl
                        for j in range(2):
                            nc.tensor.matmul(
                                psa[64 * j:64 * j + 32, 0:mh2],
                                lhsT=xcur[:, t * BC:(t + 1) * BC],
                                rhs=atb_r[:, tl, j * mh2:(j + 1) * mh2],
                                start=(t == 0), stop=(t == nch - 1),
                                tile_position=(0, 64 * j))

            # ---------------- Euler steps ----------------
            with tc.tile_pool(name="psf", bufs=2, space="PSUM") as psf, \
                 tc.tile_pool(name="ew", bufs=2) as ew:
                for k in range(q_steps):
                    if k > 0:
                        psa = psmm.tile([128, mh2], F32)
                        for t in range(nch):
                            atb_r = atbs[t // gch].rearrange(
                                "p (t m) -> p t m", m=m_loc)
                            for j in range(2):
                                nc.tensor.matmul(
                                    psa[64 * j:64 * j + 32, 0:mh2],
                                    lhsT=xcur[:, t * BC:(t + 1) * BC],
                                    rhs=atb_r[:, t % gch, j * mh2:(j + 1) * mh2],
                                    start=(t == 0), stop=(t == nch - 1),
                                    tile_position=(0, 64 * j))
                    # evict -> coupT [32, m_loc] f32, transpose -> [m, bc']
                    coupT = ew.tile([32, m_loc], F32, tag="coupT")
                    nc.vector.tensor_copy(coupT[:, 0:mh2], psa[0:32, :])
                    nc.vector.tensor_copy(coupT[:, mh2:m_loc], psa[64:96, :])
                    psb = psf.tile([128, fw], F32)
                    for mc in range(mch):
                        nc.tensor.transpose(psb[:, mc * BC:(mc + 1) * BC],
                                            coupT[:, mc * 128:(mc + 1) * 128],
                                            id32_sb)
                    psbv = psb.rearrange("p (mh eo h) -> p mh eo h", eo=2, h=HBC)
                    yev = y_e.rearrange("p (mh h) -> p mh h", h=HBC)
                    yov = y_o.rearrange("p (mh h) -> p mh h", h=HBC)
                    # update chain (even/odd halves, contiguous [128, hw])
                    ye = ew.tile([128, hw], F32, tag="ye")
                    nc.vector.tensor_add(
                        ye.rearrange("p (mh h) -> p mh h", h=HBC),
                        psbv[:, :, 0], yev)
                    yo = ew.tile([128, hw], F32, tag="yo")
                    nc.vector.tensor_add(
                        yo.rearrange("p (mh h) -> p mh h", h=HBC),
                        psbv[:, :, 1], yov)
                    se = ew.tile([128, hw], F32, tag="se")
                    nc.vector.tensor_mul(se, xe, ye)
                    so = ew.tile([128, hw], F32, tag="so")
                    nc.vector.tensor_mul(so, xo, yo)
                    sim = ew.tile([128, hw], F32, tag="sim")
                    nc.vector.tensor_add(sim, se, so)
                    # even half: xne = xe + g*(ye + w_e*xo - sim*xe)
                    u = ew.tile([128, hw], F32, tag="u")
                    nc.vector.tensor_mul(u, omg_sb[:, 0:hw], xo)
                    nc.vector.tensor_add(u, ye, u)
                    w = ew.tile([128, hw], F32, tag="w")
                    nc.vector.tensor_mul(w, sim, xe)
                    nc.vector.tensor_sub(u, u, w)
                    xne = ew.tile([128, hw], F32, tag="xne")
                    nc.vector.scalar_tensor_tensor(
                        out=xne, in0=u, scalar=gam_sb, in1=xe,
                        op0=ALU.mult, op1=ALU.add)
                    # odd half: xno = xo + g*(yo - w_o*xe - sim*xo)
                    v = ew.tile([128, hw], F32, tag="v")
                    nc.vector.tensor_mul(v, omg_sb[:, hw:2 * hw], xe)
                    nc.vector.tensor_add(v, yo, v)
                    w2 = ew.tile([128, hw], F32, tag="w2")
                    nc.vector.tensor_mul(w2, sim, xo)
                    nc.vector.tensor_sub(v, v, w2)
                    xno = ew.tile([128, hw], F32, tag="xno")
                    nc.vector.scalar_tensor_tensor(
                        out=xno, in0=v, scalar=gam_sb, in1=xo,
                        op0=ALU.mult, op1=ALU.add)
                    # renormalize pairs
                    t2 = ew.tile([128, hw], F32, tag="t2")
                    nc.scalar.activation(out=t2, in_=xne, func=ACTF.Square)
                    u3 = ew.tile([128, hw], F32, tag="u3")
                    nc.scalar.activation(out=u3, in_=xno, func=ACTF.Square)
                    ss = ew.tile([128, hw], F32, tag="ss")
                    nc.vector.tensor_add(ss, t2, u3)
                    nr = ew.tile([128, hw], F32, tag="nr")
                    nc.scalar.activation(out=nr, in_=ss, func=ACTF.Sqrt,
                                         bias=eps12_sb, scale=1.0)
                    rr = ew.tile([128, hw], F32, tag="rr")
                    nc.vector.reciprocal_approx_fast(out=rr, in_=nr)
                    xe2 = state.tile([128, hw], F32, tag="xe")
                    nc.vector.tensor_mul(xe2, xne, rr)
                    xo2 = state.tile([128, hw], F32, tag="xo")
                    nc.vector.tensor_mul(xo2, xno, rr)
                    xe, xo = xe2, xo2
                    # exchange slab (skip on last step)
                    if k < q_steps - 1:
                        sl = state.tile([128, fw], BF16, tag="sl")
                        slv = sl.rearrange("p (mh eo h) -> p mh eo h",
                                           eo=2, h=HBC)
                        nc.vector.tensor_copy(
                            slv[:, :, 0],
                            xe.rearrange("p (mh h) -> p mh h", h=HBC))
                        nc.vector.tensor_copy(
                            slv[:, :, 1],
                            xo.rearrange("p (mh h) -> p mh h", h=HBC))
                        xnew = state.tile([128, nch * BC], BF16, tag="xcur")
                        exchange(sl, xnew)
                        xcur = xnew
                    # stash into the output accumulator (off critical path)
                    nc.scalar.copy(out=oa_r[:, k, :, 0],
                                   in_=xe.rearrange("p (mh h) -> p mh h", h=HBC))
                    nc.scalar.copy(out=oa_r[:, k, :, 1],
                                   in_=xo.rearrange("p (mh h) -> p mh h", h=HBC))

            nc.sync.dma_start(out=out_d, in_=outacc)

    nc.compile()
    nc.m = get_hw_module(nc.m)
    return nc


def _bc_perm():
    """Row order (eo, b, g) -> natural row index b*C + (2g+eo)."""
    perm = []
    for eo in range(2):
        for b in range(B):
            for g in range(C // 2):
                perm.append(b * C + 2 * g + eo)
    return np.array(perm, np.int64)


def make_inputs(x, c, sc, gn_w, gn_b, conn_w, omg_param, gamma,
                n=N_FULL, ncores=N_CORES):
    """Host-side marshalling: per-core input dicts."""
    m_loc = n // ncores
    bf16 = ml_dtypes.bfloat16
    perm = _bc_perm()

    x_nat = x.reshape(BC, n)
    c_nat = np.ascontiguousarray(c.reshape(BC, n), dtype=np.float32)
    gnw_i = np.ascontiguousarray(
        np.tile(gn_w.astype(np.float32), B)[perm][:, None])
    gnb_i = np.ascontiguousarray(
        np.tile(gn_b.astype(np.float32), B)[perm][:, None])

    omg = np.abs(omg_param.astype(np.float32)[:, 0])  # [C//2]
    half = np.tile(omg, B)          # [(b g)] = 16
    # [:hw] = +w tiled per (mh, b, g); [hw:] = -w
    pos = np.tile(half, m_loc // 128)
    omg_i = np.ascontiguousarray(
        np.concatenate([pos, -pos])[None, :].astype(np.float32))

    gam_i = np.asarray(gamma, np.float32).reshape(1, 1)

    sel2 = np.zeros((128, BC), np.float32)
    for p in range(128):
        a = p // 4  # channel-row (b, c) natural
        for j in range(BC):
            nat = perm[j]
            if a // 2 == nat // 2:
                sel2[p, j] = 1.0 / 8.0
    id32 = np.eye(32, dtype=np.float32)
    id128 = np.eye(128).astype(bf16)

    shared = dict(c_nat=c_nat, gnw_i=gnw_i, gnb_i=gnb_i,
                  omg_i=omg_i, gam_i=gam_i, sel2_i=sel2,
                  id32_i=id32, id128_i=id128)
    in_maps = []
    for r in range(ncores):
        sl = slice(r * m_loc, (r + 1) * m_loc)
        in_maps.append(dict(
            shared,
            sc_s=np.ascontiguousarray(sc[0, sl, :], dtype=np.float32),
            cw_s=np.ascontiguousarray(conn_w[sl, :], dtype=np.float32),
            x_slab=np.ascontiguousarray(x_nat[perm, sl]),
            c_slab=np.ascontiguousarray(c_nat[perm, sl]),
        ))
    return in_maps


def assemble_output(outs, n=N_FULL, ncores=N_CORES, q_steps=Q_STEPS):
    """outs: list of per-core out_d [128, q*fw] -> [Q, B, N, C] f32."""
    m_loc = n // ncores
    mch = m_loc // 128
    full = np.empty((q_steps, B, n, C), np.float32)
    for r in range(ncores):
        a = outs[r].reshape(128, q_steps, mch, 2, B, C // 2)
        # -> [k, b, mh, p, g, eo] -> m = mh*128+p, c = 2g+eo
        a = a.transpose(1, 4, 2, 0, 5, 3)
        full[:, :, r * m_loc:(r + 1) * m_loc, :] = a.reshape(
            q_steps, B, m_loc, C)
    return full


_PROGRAM_CACHE = {}


def get_program(n=N_FULL, ncores=N_CORES, q_steps=Q_STEPS):
    key = (n, ncores, q_steps)
    if key not in _PROGRAM_CACHE:
        _PROGRAM_CACHE[key] = build_program(n, ncores, q_steps)
    return _PROGRAM_CACHE[key]


def kernel(x, c, sc, gn_w, gn_b, conn_w, omg_param, gamma, Q):
    assert int(Q) == Q_STEPS
    x = np.asarray(x); c = np.asarray(c); sc = np.asarray(sc)
    gn_w = np.asarray(gn_w); gn_b = np.asarray(gn_b)
    conn_w = np.asarray(conn_w); omg_param = np.asarray(omg_param)
    gamma = np.asarray(gamma)
    n = x.shape[2]
    nc = get_program(n, N_CORES, Q_STEPS)
    in_maps = make_inputs(x, c, sc, gn_w, gn_b, conn_w, omg_param, gamma,
                          n=n, ncores=N_CORES)
    res = run_bass_kernel_spmd(nc, in_maps, core_ids=list(range(N_CORES)))
    outs = [res.results[r]["out_d"] for r in range(N_CORES)]
    return assemble_output(outs, n=n)


# revision 6
# speedup vs baseline: 123.2509x; 1.0139x over previous
"""KMLayer (Kuramoto oscillator layer) on 8 Trainium2 NeuronCores via Bass/Tile.

Strategy (row-sharded, output-node parallel), v2:
  - A = sc[0] * conn_w  [N,N] row-sharded: core r owns rows m in
    [r*M_LOC, (r+1)*M_LOC).  Built on-device from 1MB streamed sc/cw slabs,
    transposed through the PE array, kept RESIDENT in SBUF as bf16 A^T
    [n-partition, m-free] (16 MB/core) in 4 column-group tiles so step-0
    matmuls overlap the build DMA.
  - Channel order is permuted host-side to (eo, b, g) ("even-first") so the
    per-step update runs on contiguous even/odd half tiles with no strided
    pair reductions.
  - Each Euler step: x-stationary matmuls over the resident A^T shard
    (2-way PE column tiling, N=512), PE transpose back to [m, bc], a lean
    DVE/ACT update chain, then the new local slab (bf16, p-major layout)
    is AllGather'd; the gather-in DMA reads 512-byte lines.
  - Outputs accumulate in SBUF; one 1MB DMA at the end; host reassembles.
State is carried in fp32; matmul operands (A, gathered X) are bf16.
"""

import numpy as np
import ml_dtypes

import concourse.bass as bass
import concourse.mybir as mybir
import concourse.tile as tile
from concourse import bacc
from concourse.bass_utils import run_bass_kernel_spmd
from concourse.bass_interp import get_hw_module

F32 = mybir.dt.float32
BF16 = mybir.dt.bfloat16
ALU = mybir.AluOpType
ACTF = mybir.ActivationFunctionType
AXX = mybir.AxisListType.X

N_CORES = 8
B, C, N_FULL = 2, 16, 8192
BC = B * C  # 32
HBC = BC // 2  # 16: even (or odd) half of the channel axis
Q_STEPS = 8
GN_EPS = 1e-5
NRM_EPS2 = 1e-12  # guards Rsqrt(ss); ref adds 1e-6 to the norm instead
N_GROUPS = 4      # A^T column-group tiles (build/step-0 overlap granularity)


def _bcast(ap, parts):
    """Partition-broadcast view of a [1, f] DRAM AP -> [parts, f]."""
    return bass.AP(tensor=ap.tensor, offset=ap.offset, ap=[[0, parts]] + list(ap.ap[1:]))


def build_program(n=N_FULL, ncores=N_CORES, q_steps=Q_STEPS):
    m_loc = n // ncores            # rows owned per core
    mch = m_loc // 128             # 128-row chunks per core (8)
    nch = n // 128                 # 128-col contraction chunks (64)
    gch = nch // N_GROUPS          # n-chunks per A^T group (16)
    mh2 = m_loc // 2               # m-range per PE column-tile group (512)
    rg = [list(range(ncores))]
    fw = mch * BC                  # per-node free width (256)
    hw = fw // 2                   # even/odd half width (128)

    nc = bacc.Bacc("TRN2", target_bir_lowering=False, debug=False,
                   enable_asserts=False, num_devices=ncores)

    # ---- I/O ----
    sc_s = nc.dram_tensor("sc_s", [m_loc, n], F32, kind="ExternalInput").ap()
    cw_s = nc.dram_tensor("cw_s", [m_loc, n], F32, kind="ExternalInput").ap()
    c_nat = nc.dram_tensor("c_nat", [BC, n], F32, kind="ExternalInput").ap()
    x_slab = nc.dram_tensor("x_slab", [BC, m_loc], F32, kind="ExternalInput").ap()
    c_slab = nc.dram_tensor("c_slab", [BC, m_loc], F32, kind="ExternalInput").ap()
    gnw_i = nc.dram_tensor("gnw_i", [BC, 1], F32, kind="ExternalInput").ap()
    gnb_i = nc.dram_tensor("gnb_i", [BC, 1], F32, kind="ExternalInput").ap()
    omg_i = nc.dram_tensor("omg_i", [1, 2 * hw], F32, kind="ExternalInput").ap()
    gam_i = nc.dram_tensor("gam_i", [1, 1], F32, kind="ExternalInput").ap()
    sel2_i = nc.dram_tensor("sel2_i", [128, BC], F32, kind="ExternalInput").ap()
    id32_i = nc.dram_tensor("id32_i", [32, 32], F32, kind="ExternalInput").ap()
    id128_i = nc.dram_tensor("id128_i", [128, 128], BF16, kind="ExternalInput").ap()
    # [p, k, mh, eo, b, g] f32 -- host reassembles
    out_d = nc.dram_tensor("out_d", [128, q_steps * fw], F32,
                           kind="ExternalOutput").ap()

    with tile.TileContext(nc) as tc:
        with tc.tile_pool(name="consts", bufs=1) as consts, \
             tc.tile_pool(name="atbp", bufs=1) as atbp, \
             tc.tile_pool(name="state", bufs=2) as state, \
             tc.tile_pool(name="outp", bufs=1) as outp, \
             tc.tile_pool(name="agd", bufs=2, space="DRAM") as agd, \
             tc.tile_pool(name="psmm", bufs=1, space="PSUM") as psmm:

            # ---------------- constants ----------------
            sel2_sb = consts.tile([128, BC], F32)
            nc.sync.dma_start(out=sel2_sb, in_=sel2_i)
            id32_sb = consts.tile([32, 32], F32)
            nc.sync.dma_start(out=id32_sb, in_=id32_i)
            id128_sb = consts.tile([128, 128], BF16)
            nc.sync.dma_start(out=id128_sb, in_=id128_i)
            gnw_sb = consts.tile([BC, 1], F32)
            nc.sync.dma_start(out=gnw_sb, in_=gnw_i)
            gnb_sb = consts.tile([BC, 1], F32)
            nc.sync.dma_start(out=gnb_sb, in_=gnb_i)
            omg_sb = consts.tile([128, 2 * hw], F32)   # [:, :hw]=+w, [:, hw:]=-w
            nc.sync.dma_start(out=omg_sb, in_=_bcast(omg_i, 128))
            gam_sb = consts.tile([128, 1], F32)
            nc.sync.dma_start(out=gam_sb, in_=_bcast(gam_i, 128))
            eps5_sb = consts.tile([BC, 1], F32)
            nc.vector.memset(eps5_sb, GN_EPS)
            eps12_sb = consts.tile([128, 1], F32)
            nc.vector.memset(eps12_sb, NRM_EPS2)

            # persistent A^T shard: 4 group tiles, each
            # [128 n_lo, (gch n_hi)(m_loc) free] bf16
            atbs = []
            for g in range(N_GROUPS):
                atb_g = atbp.tile([128, gch * m_loc], BF16, tag=f"atb{g}")
                atbs.append(atb_g)

            # state: local even/odd halves (f32) + gathered full x (bf16)
            xe = state.tile([128, hw], F32, tag="xe")
            xo = state.tile([128, hw], F32, tag="xo")
            xcur = state.tile([128, nch * BC], BF16, tag="xcur")

            # output accumulator [p, (k mh eo bg)] f32
            outacc = outp.tile([128, q_steps * fw], F32)
            oa_r = outacc.rearrange("p (k mh eo bg) -> p k mh eo bg",
                                    k=q_steps, mh=mch, eo=2)

            def exchange(sl_tile, xcur_dst):
                """slab [128, fw] bf16 -> AllGather -> xcur_dst [128, nch*BC]."""
                agi = agd.tile([128, fw], BF16, tag="agi")
                nc.sync.dma_start(out=agi, in_=sl_tile)
                ago = agd.tile([ncores * 128, fw], BF16, tag="ago")
                nc.gpsimd.collective_compute(
                    "AllGather", ALU.bypass, replica_groups=rg,
                    ins=[agi.opt()], outs=[ago.opt()])
                nc.sync.dma_start(
                    out=xcur_dst.rearrange("p (r f) -> p r f", r=ncores),
                    in_=ago.rearrange("(r p) f -> p r f", p=128))

            # ---------------- init: groupnorm stats + y + x0 ----------------
            with tc.tile_pool(name="initp", bufs=1) as initp, \
                 tc.tile_pool(name="psinit", bufs=2, space="PSUM") as psinit:
                # -- groupnorm statistics over full c (natural order) --
                c128 = initp.tile([128, n // 4], F32, tag="ibig")
                nc.sync.dma_start(out=c128,
                                  in_=c_nat.rearrange("a (q m) -> (a q) m", q=4))
                fsub = n // 4
                nsub = 1
                while fsub > 512:
                    assert fsub % 2 == 0
                    fsub //= 2
                    nsub *= 2
                stats = initp.tile([128, nsub, 6], F32)
                c128v = c128.rearrange("p (s m) -> p s m", s=nsub)
                for s in range(nsub):
                    nc.vector.bn_stats(out=stats[:, s, :], in_=c128v[:, s, :])
                mv = initp.tile([128, 2], F32)
                nc.vector.bn_aggr(out=mv, in_=stats)
                # mv[:,1] <- E[x^2] = mean^2 + var
                nc.vector.scalar_tensor_tensor(
                    out=mv[:, 1:2], in0=mv[:, 0:1], scalar=mv[:, 0:1],
                    in1=mv[:, 1:2], op0=ALU.mult, op1=ALU.add)
                ps_s = psinit.tile([32, 2], F32, tag="ps_small")
                nc.tensor.matmul(ps_s, lhsT=sel2_sb, rhs=mv, start=True, stop=True)
                mvg = initp.tile([BC, 2], F32)
                nc.vector.tensor_copy(mvg, ps_s)
                mu2 = initp.tile([BC, 1], F32)
                nc.vector.tensor_mul(mu2, mvg[:, 0:1], mvg[:, 0:1])
                var32 = initp.tile([BC, 1], F32)
                nc.vector.tensor_sub(var32, mvg[:, 1:2], mu2)
                sd32 = initp.tile([BC, 1], F32)
                nc.scalar.activation(out=sd32, in_=var32, func=ACTF.Sqrt,
                                     bias=eps5_sb, scale=1.0)
                rstd = initp.tile([BC, 1], F32)
                nc.vector.reciprocal(out=rstd, in_=sd32)
                scl32 = initp.tile([BC, 1], F32)
                nc.vector.tensor_mul(scl32, rstd, gnw_sb)
                nmu = initp.tile([BC, 1], F32)
                nc.vector.tensor_scalar_mul(nmu, mvg[:, 0:1], -1.0)
                bia32 = initp.tile([BC, 1], F32)
                nc.vector.scalar_tensor_tensor(
                    out=bia32, in0=nmu, scalar=scl32, in1=gnb_sb,
                    op0=ALU.mult, op1=ALU.add)

                # -- y (normalized c) for the local slab, transposed --
                # c_slab rows are already host-permuted to (eo, b, g) order
                csl = initp.tile([BC, m_loc], F32, tag="isl")
                nc.sync.dma_start(out=csl, in_=c_slab)
                ysl = initp.tile([BC, m_loc], F32, tag="isl2")
                nc.scalar.activation(out=ysl, in_=csl, func=ACTF.Identity,
                                     bias=bia32, scale=scl32)
                ysl_bf = consts.tile([32, m_loc], BF16)
                nc.scalar.copy(out=ysl_bf, in_=ysl)

                # -- x0 local slab: transpose + pair-normalize (even/odd) --
                xsl = initp.tile([BC, m_loc], F32, tag="isl")
                nc.sync.dma_start(out=xsl, in_=x_slab)
                ps_x = psinit.tile([128, fw], F32, tag="ps_y")
                for mc in range(mch):
                    nc.tensor.transpose(ps_x[:, mc * BC:(mc + 1) * BC],
                                        xsl[:, mc * 128:(mc + 1) * 128], id32_sb)
                x0f = initp.tile([128, fw], F32)
                nc.vector.tensor_copy(x0f, ps_x)
                x0v = x0f.rearrange("p (mh eo h) -> p mh eo h", eo=2, h=HBC)
                xev = xe.rearrange("p (mh h) -> p mh h", h=HBC)
                xov = xo.rearrange("p (mh h) -> p mh h", h=HBC)
                t2 = initp.tile([128, hw], F32, tag="t2")
                u3 = initp.tile([128, hw], F32, tag="u3")
                nc.scalar.activation(out=t2.rearrange("p (mh h) -> p mh h", h=HBC),
                                     in_=x0v[:, :, 0], func=ACTF.Square)
                nc.scalar.activation(out=u3.rearrange("p (mh h) -> p mh h", h=HBC),
                                     in_=x0v[:, :, 1], func=ACTF.Square)
                ss = initp.tile([128, hw], F32, tag="ss")
                nc.vector.tensor_add(ss, t2, u3)
                nr = initp.tile([128, hw], F32, tag="nr")
                nc.scalar.activation(out=nr, in_=ss, func=ACTF.Sqrt,
                                     bias=eps12_sb, scale=1.0)
                rr = initp.tile([128, hw], F32, tag="rr")
                nc.vector.reciprocal_approx_fast(out=rr, in_=nr)
                nc.vector.tensor_mul(xev, x0v[:, :, 0], rr.rearrange(
                    "p (mh h) -> p mh h", h=HBC))
                nc.vector.tensor_mul(xov, x0v[:, :, 1], rr.rearrange(
                    "p (mh h) -> p mh h", h=HBC))
                # slab for AG#0
                sl0 = state.tile([128, fw], BF16, tag="sl")
                sl0v = sl0.rearrange("p (mh eo h) -> p mh eo h", eo=2, h=HBC)
                nc.vector.tensor_copy(sl0v[:, :, 0], xev)
                nc.vector.tensor_copy(sl0v[:, :, 1], xov)
                exchange(sl0, xcur)

            # ---------------- build A^T shard (+ step-0 matmuls) -----------
            # psa: 2-way column tiling, group j -> psum partitions 64j..64j+31,
            # cols 0..511 (m-half j)
            psa = psmm.tile([128, mh2], F32)
            piece = 2048
            with tc.tile_pool(name="bstage", bufs=3) as bstage, \
                 tc.tile_pool(name="bprod", bufs=2) as bprod, \
                 tc.tile_pool(name="pst", bufs=2, space="PSUM") as pst:
                for g in range(N_GROUPS):
                    atb = atbs[g]
                    atb_r = atb.rearrange("p (t m) -> p t m", m=m_loc)
                    for j in range(mch):
                        scp = bstage.tile([128, piece], F32, tag="scp")
                        nc.sync.dma_start(
                            out=scp,
                            in_=sc_s[j * 128:(j + 1) * 128,
                                     g * piece:(g + 1) * piece])
                        cwp = bstage.tile([128, piece], F32, tag="cwp")
                        nc.sync.dma_start(
                            out=cwp,
                            in_=cw_s[j * 128:(j + 1) * 128,
                                     g * piece:(g + 1) * piece])
                        prod = bprod.tile([128, piece], BF16, tag="prod")
                        nc.vector.tensor_mul(prod, scp, cwp)
                        for half in range(2):
                            pt = pst.tile([128, 8 * 128], BF16, tag="pt")
                            for tt in range(8):
                                t = half * 8 + tt
                                nc.tensor.transpose(
                                    pt[:, tt * 128:(tt + 1) * 128],
                                    prod[:, t * 128:(t + 1) * 128], id128_sb)
                            dst = atb_r[:, half * 8:(half + 1) * 8,
                                        j * 128:(j + 1) * 128]
                            nc.scalar.copy(
                                out=dst,
                                in_=pt.rearrange("p (t m) -> p t m", m=128))
                    # step-0 matmuls for this group's n-chunks
                    # (the constant y enters the accumulation as an extra
                    # K=32 identity chunk: psa += I.T @ y^T)
                    if g == 0:
                        for j in range(2):
                            nc.tensor.matmul(
                                psa[64 * j:64 * j + 32, 0:mh2],
                                lhsT=id128_sb[0:32, 0:32],
                                rhs=ysl_bf[:, j * mh2:(j + 1) * mh2],
                                start=True, stop=False,
                                tile_position=(0, 64 * j))
                    for tl in range(gch):
                        t = g * gch + tl
                        for j in range(2):
                            nc.tensor.matmul(
                                psa[64 * j:64 * j + 32, 0:mh2],
                                lhsT=xcur[:, t * BC:(t + 1) * BC],
                                rhs=atb_r[:, tl, j * mh2:(j + 1) * mh2],
                                start=False, stop=(t == nch - 1),
                                tile_position=(0, 64 * j))

            # ---------------- Euler steps ----------------
            with tc.tile_pool(name="psf", bufs=2, space="PSUM") as psf, \
                 tc.tile_pool(name="ew", bufs=2) as ew:
                for k in range(q_steps):
                    if k > 0:
                        psa = psmm.tile([128, mh2], F32)
                        for j in range(2):
                            nc.tensor.matmul(
                                psa[64 * j:64 * j + 32, 0:mh2],
                                lhsT=id128_sb[0:32, 0:32],
                                rhs=ysl_bf[:, j * mh2:(j + 1) * mh2],
                                start=True, stop=False,
                                tile_position=(0, 64 * j))
                        for t in range(nch):
                            atb_r = atbs[t // gch].rearrange(
                                "p (t m) -> p t m", m=m_loc)
                            for j in range(2):
                                nc.tensor.matmul(
                                    psa[64 * j:64 * j + 32, 0:mh2],
                                    lhsT=xcur[:, t * BC:(t + 1) * BC],
                                    rhs=atb_r[:, t % gch, j * mh2:(j + 1) * mh2],
                                    start=False, stop=(t == nch - 1),
                                    tile_position=(0, 64 * j))
                    # evict -> coupT [32, m_loc] f32, transpose -> [m, bc']
                    coupT = ew.tile([32, m_loc], F32, tag="coupT")
                    nc.vector.tensor_copy(coupT[:, 0:mh2], psa[0:32, :])
                    nc.vector.tensor_copy(coupT[:, mh2:m_loc], psa[64:96, :])
                    psb = psf.tile([128, fw], F32)
                    for mc in range(mch):
                        nc.tensor.transpose(psb[:, mc * BC:(mc + 1) * BC],
                                            coupT[:, mc * 128:(mc + 1) * 128],
                                            id32_sb)
                    psbv = psb.rearrange("p (mh eo h) -> p mh eo h", eo=2, h=HBC)
                    pe_v = psbv[:, :, 0]
                    po_v = psbv[:, :, 1]
                    xev_ = xe.rearrange("p (mh h) -> p mh h", h=HBC)
                    xov_ = xo.rearrange("p (mh h) -> p mh h", h=HBC)
                    # update chain; psb already holds yt = coup + y
                    se = ew.tile([128, hw], F32, tag="se")
                    nc.vector.tensor_mul(
                        se.rearrange("p (mh h) -> p mh h", h=HBC), xev_, pe_v)
                    so = ew.tile([128, hw], F32, tag="so")
                    nc.vector.tensor_mul(
                        so.rearrange("p (mh h) -> p mh h", h=HBC), xov_, po_v)
                    sim = ew.tile([128, hw], F32, tag="sim")
                    nc.vector.tensor_add(sim, se, so)
                    # even half: xne = xe + g*(yt_e + w_e*xo - sim*xe)
                    u = ew.tile([128, hw], F32, tag="u")
                    nc.vector.tensor_mul(u, omg_sb[:, 0:hw], xo)
                    nc.vector.tensor_add(
                        u.rearrange("p (mh h) -> p mh h", h=HBC),
                        pe_v, u.rearrange("p (mh h) -> p mh h", h=HBC))
                    w = ew.tile([128, hw], F32, tag="w")
                    nc.vector.tensor_mul(w, sim, xe)
                    nc.vector.tensor_sub(u, u, w)
                    xne = ew.tile([128, hw], F32, tag="xne")
                    nc.vector.scalar_tensor_tensor(
                        out=xne, in0=u, scalar=gam_sb, in1=xe,
                        op0=ALU.mult, op1=ALU.add)
                    # odd half: xno = xo + g*(yt_o - w_o*xe - sim*xo)
                    v = ew.tile([128, hw], F32, tag="v")
                    nc.vector.tensor_mul(v, omg_sb[:, hw:2 * hw], xe)
                    nc.vector.tensor_add(
                        v.rearrange("p (mh h) -> p mh h", h=HBC),
                        po_v, v.rearrange("p (mh h) -> p mh h", h=HBC))
                    w2 = ew.tile([128, hw], F32, tag="w2")
                    nc.vector.tensor_mul(w2, sim, xo)
                    nc.vector.tensor_sub(v, v, w2)
                    xno = ew.tile([128, hw], F32, tag="xno")
                    nc.vector.scalar_tensor_tensor(
                        out=xno, in0=v, scalar=gam_sb, in1=xo,
                        op0=ALU.mult, op1=ALU.add)
                    # renormalize pairs
                    t2 = ew.tile([128, hw], F32, tag="t2")
                    nc.scalar.activation(out=t2, in_=xne, func=ACTF.Square)
                    u3 = ew.tile([128, hw], F32, tag="u3")
                    nc.scalar.activation(out=u3, in_=xno, func=ACTF.Square)
                    ss = ew.tile([128, hw], F32, tag="ss")
                    nc.vector.tensor_add(ss, t2, u3)
                    nr = ew.tile([128, hw], F32, tag="nr")
                    nc.scalar.activation(out=nr, in_=ss, func=ACTF.Sqrt,
                                         bias=eps12_sb, scale=1.0)
                    rr = ew.tile([128, hw], F32, tag="rr")
                    nc.vector.reciprocal_approx_fast(out=rr, in_=nr)
                    xe2 = state.tile([128, hw], F32, tag="xe")
                    nc.vector.tensor_mul(xe2, xne, rr)
                    xo2 = state.tile([128, hw], F32, tag="xo")
                    nc.vector.tensor_mul(xo2, xno, rr)
                    xe, xo = xe2, xo2
                    # PE-warming filler chain: occupies the tensor engine
                    # through the DVE + AllGather window so HAM keeps the PE
                    # at 2.4 GHz (tag-shared PSUM slot serializes the chain
                    # between this step's evictions and next step's matmuls)
                    if k < q_steps - 1:
                        fps = psmm.tile([128, mh2], F32, tag="psa")
                        n_fill = 68
                        for fi in range(n_fill):
                            nc.tensor.matmul(
                                fps[0:32, 0:mh2],
                                lhsT=id128_sb[:, 0:32],
                                rhs=atbs[0].rearrange(
                                    "p (t m) -> p t m", m=m_loc)[:, 0, 0:mh2],
                                start=(fi == 0), stop=(fi == n_fill - 1),
                                tile_position=(0, 0))
                    # exchange slab (skip on last step)
                    if k < q_steps - 1:
                        sl = state.tile([128, fw], BF16, tag="sl")
                        slv = sl.rearrange("p (mh eo h) -> p mh eo h",
                                           eo=2, h=HBC)
                        nc.vector.tensor_copy(
                            slv[:, :, 0],
                            xe.rearrange("p (mh h) -> p mh h", h=HBC))
                        nc.vector.tensor_copy(
                            slv[:, :, 1],
                            xo.rearrange("p (mh h) -> p mh h", h=HBC))
                        xnew = state.tile([128, nch * BC], BF16, tag="xcur")
                        exchange(sl, xnew)
                        xcur = xnew
                    # stash into the output accumulator (off critical path)
                    nc.scalar.copy(out=oa_r[:, k, :, 0],
                                   in_=xe.rearrange("p (mh h) -> p mh h", h=HBC))
                    nc.scalar.copy(out=oa_r[:, k, :, 1],
                                   in_=xo.rearrange("p (mh h) -> p mh h", h=HBC))

            nc.sync.dma_start(out=out_d, in_=outacc)

    nc.compile()
    nc.m = get_hw_module(nc.m)
    return nc


def _bc_perm():
    """Row order (eo, b, g) -> natural row index b*C + (2g+eo)."""
    perm = []
    for eo in range(2):
        for b in range(B):
            for g in range(C // 2):
                perm.append(b * C + 2 * g + eo)
    return np.array(perm, np.int64)


def make_inputs(x, c, sc, gn_w, gn_b, conn_w, omg_param, gamma,
                n=N_FULL, ncores=N_CORES):
    """Host-side marshalling: per-core input dicts."""
    m_loc = n // ncores
    bf16 = ml_dtypes.bfloat16
    perm = _bc_perm()

    x_nat = x.reshape(BC, n)
    c_nat = np.ascontiguousarray(c.reshape(BC, n), dtype=np.float32)
    gnw_i = np.ascontiguousarray(
        np.tile(gn_w.astype(np.float32), B)[perm][:, None])
    gnb_i = np.ascontiguousarray(
        np.tile(gn_b.astype(np.float32), B)[perm][:, None])

    omg = np.abs(omg_param.astype(np.float32)[:, 0])  # [C//2]
    half = np.tile(omg, B)          # [(b g)] = 16
    # [:hw] = +w tiled per (mh, b, g); [hw:] = -w
    pos = np.tile(half, m_loc // 128)
    omg_i = np.ascontiguousarray(
        np.concatenate([pos, -pos])[None, :].astype(np.float32))

    gam_i = np.asarray(gamma, np.float32).reshape(1, 1)

    sel2 = np.zeros((128, BC), np.float32)
    for p in range(128):
        a = p // 4  # channel-row (b, c) natural
        for j in range(BC):
            nat = perm[j]
            if a // 2 == nat // 2:
                sel2[p, j] = 1.0 / 8.0
    id32 = np.eye(32, dtype=np.float32)
    id128 = np.eye(128).astype(bf16)

    shared = dict(c_nat=c_nat, gnw_i=gnw_i, gnb_i=gnb_i,
                  omg_i=omg_i, gam_i=gam_i, sel2_i=sel2,
                  id32_i=id32, id128_i=id128)
    in_maps = []
    for r in range(ncores):
        sl = slice(r * m_loc, (r + 1) * m_loc)
        in_maps.append(dict(
            shared,
            sc_s=np.ascontiguousarray(sc[0, sl, :], dtype=np.float32),
            cw_s=np.ascontiguousarray(conn_w[sl, :], dtype=np.float32),
            x_slab=np.ascontiguousarray(x_nat[perm, sl]),
            c_slab=np.ascontiguousarray(c_nat[perm, sl]),
        ))
    return in_maps


def assemble_output(outs, n=N_FULL, ncores=N_CORES, q_steps=Q_STEPS):
    """outs: list of per-core out_d [128, q*fw] -> [Q, B, N, C] f32."""
    m_loc = n // ncores
    mch = m_loc // 128
    full = np.empty((q_steps, B, n, C), np.float32)
    for r in range(ncores):
        a = outs[r].reshape(128, q_steps, mch, 2, B, C // 2)
        # -> [k, b, mh, p, g, eo] -> m = mh*128+p, c = 2g+eo
        a = a.transpose(1, 4, 2, 0, 5, 3)
        full[:, :, r * m_loc:(r + 1) * m_loc, :] = a.reshape(
            q_steps, B, m_loc, C)
    return full


_PROGRAM_CACHE = {}


def get_program(n=N_FULL, ncores=N_CORES, q_steps=Q_STEPS):
    key = (n, ncores, q_steps)
    if key not in _PROGRAM_CACHE:
        _PROGRAM_CACHE[key] = build_program(n, ncores, q_steps)
    return _PROGRAM_CACHE[key]


def kernel(x, c, sc, gn_w, gn_b, conn_w, omg_param, gamma, Q):
    assert int(Q) == Q_STEPS
    x = np.asarray(x); c = np.asarray(c); sc = np.asarray(sc)
    gn_w = np.asarray(gn_w); gn_b = np.asarray(gn_b)
    conn_w = np.asarray(conn_w); omg_param = np.asarray(omg_param)
    gamma = np.asarray(gamma)
    n = x.shape[2]
    nc = get_program(n, N_CORES, Q_STEPS)
    in_maps = make_inputs(x, c, sc, gn_w, gn_b, conn_w, omg_param, gamma,
                          n=n, ncores=N_CORES)
    res = run_bass_kernel_spmd(nc, in_maps, core_ids=list(range(N_CORES)))
    outs = [res.results[r]["out_d"] for r in range(N_CORES)]
    return assemble_output(outs, n=n)


# revision 8
# speedup vs baseline: 136.2531x; 1.1055x over previous
"""KMLayer (Kuramoto oscillator layer) on 8 Trainium2 NeuronCores via Bass/Tile.

Strategy (row-sharded, output-node parallel), v2:
  - A = sc[0] * conn_w  [N,N] row-sharded: core r owns rows m in
    [r*M_LOC, (r+1)*M_LOC).  Built on-device from 1MB streamed sc/cw slabs,
    transposed through the PE array, kept RESIDENT in SBUF as bf16 A^T
    [n-partition, m-free] (16 MB/core) in 4 column-group tiles so step-0
    matmuls overlap the build DMA.
  - Channel order is permuted host-side to (eo, b, g) ("even-first") so the
    per-step update runs on contiguous even/odd half tiles with no strided
    pair reductions.
  - Each Euler step: x-stationary matmuls over the resident A^T shard
    (2-way PE column tiling, N=512), PE transpose back to [m, bc], a lean
    DVE/ACT update chain, then the new local slab (bf16, p-major layout)
    is AllGather'd; the gather-in DMA reads 512-byte lines.
  - Outputs accumulate in SBUF; one 1MB DMA at the end; host reassembles.
State is carried in fp32; matmul operands (A, gathered X) are bf16.
"""

import numpy as np
import ml_dtypes

import concourse.bass as bass
import concourse.mybir as mybir
import concourse.tile as tile
from concourse import bacc
from concourse.bass_utils import run_bass_kernel_spmd
from concourse.bass_interp import get_hw_module

F32 = mybir.dt.float32
BF16 = mybir.dt.bfloat16
ALU = mybir.AluOpType
ACTF = mybir.ActivationFunctionType
AXX = mybir.AxisListType.X

N_CORES = 8
B, C, N_FULL = 2, 16, 8192
BC = B * C  # 32
HBC = BC // 2  # 16: even (or odd) half of the channel axis
Q_STEPS = 8
GN_EPS = 1e-5
NRM_EPS2 = 1e-12  # guards Rsqrt(ss); ref adds 1e-6 to the norm instead
N_GROUPS = 4      # A^T column-group tiles (build/step-0 overlap granularity)


def _bcast(ap, parts):
    """Partition-broadcast view of a [1, f] DRAM AP -> [parts, f]."""
    return bass.AP(tensor=ap.tensor, offset=ap.offset, ap=[[0, parts]] + list(ap.ap[1:]))


def build_program(n=N_FULL, ncores=N_CORES, q_steps=Q_STEPS):
    m_loc = n // ncores            # rows owned per core
    mch = m_loc // 128             # 128-row chunks per core (8)
    nch = n // 128                 # 128-col contraction chunks (64)
    gch = nch // N_GROUPS          # n-chunks per A^T group (16)
    mh2 = m_loc // 2               # m-range per PE column-tile group (512)
    rg = [list(range(ncores))]
    fw = mch * BC                  # per-node free width (256)
    hw = fw // 2                   # even/odd half width (128)

    nc = bacc.Bacc("TRN2", target_bir_lowering=False, debug=False,
                   enable_asserts=False, num_devices=ncores)

    # ---- I/O ----
    sc_s = nc.dram_tensor("sc_s", [m_loc, n], F32, kind="ExternalInput").ap()
    cw_s = nc.dram_tensor("cw_s", [m_loc, n], F32, kind="ExternalInput").ap()
    c_nat = nc.dram_tensor("c_nat", [BC, n], F32, kind="ExternalInput").ap()
    x_slab = nc.dram_tensor("x_slab", [BC, m_loc], F32, kind="ExternalInput").ap()
    c_slab = nc.dram_tensor("c_slab", [BC, m_loc], F32, kind="ExternalInput").ap()
    gnw_i = nc.dram_tensor("gnw_i", [BC, 1], F32, kind="ExternalInput").ap()
    gnb_i = nc.dram_tensor("gnb_i", [BC, 1], F32, kind="ExternalInput").ap()
    omg_i = nc.dram_tensor("omg_i", [1, 2 * hw], F32, kind="ExternalInput").ap()
    gam_i = nc.dram_tensor("gam_i", [1, 1], F32, kind="ExternalInput").ap()
    sel2_i = nc.dram_tensor("sel2_i", [128, BC], F32, kind="ExternalInput").ap()
    id32_i = nc.dram_tensor("id32_i", [32, 32], F32, kind="ExternalInput").ap()
    id128_i = nc.dram_tensor("id128_i", [128, 128], BF16, kind="ExternalInput").ap()
    # [p, k, mh, eo, b, g] f32 -- host reassembles
    out_d = nc.dram_tensor("out_d", [128, q_steps * fw], F32,
                           kind="ExternalOutput").ap()

    with tile.TileContext(nc) as tc:
        with tc.tile_pool(name="consts", bufs=1) as consts, \
             tc.tile_pool(name="atbp", bufs=1) as atbp, \
             tc.tile_pool(name="state", bufs=2) as state, \
             tc.tile_pool(name="outp", bufs=1) as outp, \
             tc.tile_pool(name="agd", bufs=2, space="DRAM") as agd, \
             tc.tile_pool(name="psmm", bufs=1, space="PSUM") as psmm:

            # ---------------- constants ----------------
            sel2_sb = consts.tile([128, BC], F32)
            nc.sync.dma_start(out=sel2_sb, in_=sel2_i)
            id32_sb = consts.tile([32, 32], F32)
            nc.sync.dma_start(out=id32_sb, in_=id32_i)
            id128_sb = consts.tile([128, 128], BF16)
            nc.sync.dma_start(out=id128_sb, in_=id128_i)
            gnw_sb = consts.tile([BC, 1], F32)
            nc.sync.dma_start(out=gnw_sb, in_=gnw_i)
            gnb_sb = consts.tile([BC, 1], F32)
            nc.sync.dma_start(out=gnb_sb, in_=gnb_i)
            omg_sb = consts.tile([128, 2 * hw], F32)   # [:, :hw]=+w, [:, hw:]=-w
            nc.sync.dma_start(out=omg_sb, in_=_bcast(omg_i, 128))
            gam_sb = consts.tile([128, 1], F32)
            nc.sync.dma_start(out=gam_sb, in_=_bcast(gam_i, 128))
            eps5_sb = consts.tile([BC, 1], F32)
            nc.vector.memset(eps5_sb, GN_EPS)
            eps12_sb = consts.tile([128, 1], F32)
            nc.vector.memset(eps12_sb, NRM_EPS2)

            # persistent A^T shard: 4 group tiles, each
            # [128 n_lo, (gch n_hi)(m_loc) free] bf16
            atbs = []
            for g in range(N_GROUPS):
                atb_g = atbp.tile([128, gch * m_loc], BF16, tag=f"atb{g}")
                atbs.append(atb_g)

            # state: local even/odd halves (f32) + gathered full x (bf16)
            xe = state.tile([128, hw], F32, tag="xe")
            xo = state.tile([128, hw], F32, tag="xo")
            xcur_a = state.tile([128, nch * BC // 2], BF16, tag="xcur_a")
            xcur_b = state.tile([128, nch * BC // 2], BF16, tag="xcur_b")
            xcur_h = [xcur_a, xcur_b]

            # output accumulator [p, (k mh eo bg)] f32
            outacc = outp.tile([128, q_steps * fw], F32)
            oa_r = outacc.rearrange("p (k mh eo bg) -> p k mh eo bg",
                                    k=q_steps, mh=mch, eo=2)

            def exchange(sl_tile, xcur_dsts):
                """slab [128, fw] bf16 -> AllGather -> two half tiles
                [128, nch*BC/2] (ranks 0-3 / 4-7) so matmuls can start on
                the first half while the second gathers."""
                agi = agd.tile([128, fw], BF16, tag="agi")
                nc.sync.dma_start(out=agi, in_=sl_tile)
                ago = agd.tile([ncores * 128, fw], BF16, tag="ago")
                nc.gpsimd.collective_compute(
                    "AllGather", ALU.bypass, replica_groups=rg,
                    ins=[agi.opt()], outs=[ago.opt()])
                agov = ago.rearrange("(r p) f -> p r f", p=128)
                for h_ in range(2):
                    nc.sync.dma_start(
                        out=xcur_dsts[h_].rearrange("p (r f) -> p r f",
                                                    r=ncores // 2),
                        in_=agov[:, h_ * (ncores // 2):(h_ + 1) * (ncores // 2)])

            # ---------------- init: groupnorm stats + y + x0 ----------------
            with tc.tile_pool(name="initp", bufs=1) as initp, \
                 tc.tile_pool(name="psinit", bufs=1, space="PSUM") as psinit:
                # -- groupnorm statistics over full c (natural order) --
                c128 = initp.tile([128, n // 4], F32, tag="ibig")
                nc.sync.dma_start(out=c128,
                                  in_=c_nat.rearrange("a (q m) -> (a q) m", q=4))
                fsub = n // 4
                nsub = 1
                while fsub > 512:
                    assert fsub % 2 == 0
                    fsub //= 2
                    nsub *= 2
                stats = initp.tile([128, nsub, 6], F32)
                c128v = c128.rearrange("p (s m) -> p s m", s=nsub)
                for s in range(nsub):
                    nc.vector.bn_stats(out=stats[:, s, :], in_=c128v[:, s, :])
                mv = initp.tile([128, 2], F32)
                nc.vector.bn_aggr(out=mv, in_=stats)
                # mv[:,1] <- E[x^2] = mean^2 + var
                nc.vector.scalar_tensor_tensor(
                    out=mv[:, 1:2], in0=mv[:, 0:1], scalar=mv[:, 0:1],
                    in1=mv[:, 1:2], op0=ALU.mult, op1=ALU.add)
                ps_s = psinit.tile([32, 2], F32, tag="ps_small")
                nc.tensor.matmul(ps_s, lhsT=sel2_sb, rhs=mv, start=True, stop=True)
                mvg = initp.tile([BC, 2], F32)
                nc.vector.tensor_copy(mvg, ps_s)
                mu2 = initp.tile([BC, 1], F32)
                nc.vector.tensor_mul(mu2, mvg[:, 0:1], mvg[:, 0:1])
                var32 = initp.tile([BC, 1], F32)
                nc.vector.tensor_sub(var32, mvg[:, 1:2], mu2)
                sd32 = initp.tile([BC, 1], F32)
                nc.scalar.activation(out=sd32, in_=var32, func=ACTF.Sqrt,
                                     bias=eps5_sb, scale=1.0)
                rstd = initp.tile([BC, 1], F32)
                nc.vector.reciprocal(out=rstd, in_=sd32)
                scl32 = initp.tile([BC, 1], F32)
                nc.vector.tensor_mul(scl32, rstd, gnw_sb)
                nmu = initp.tile([BC, 1], F32)
                nc.vector.tensor_scalar_mul(nmu, mvg[:, 0:1], -1.0)
                bia32 = initp.tile([BC, 1], F32)
                nc.vector.scalar_tensor_tensor(
                    out=bia32, in0=nmu, scalar=scl32, in1=gnb_sb,
                    op0=ALU.mult, op1=ALU.add)

                # -- y (normalized c) for the local slab, transposed --
                # c_slab rows are already host-permuted to (eo, b, g) order
                csl = initp.tile([BC, m_loc], F32, tag="isl")
                nc.sync.dma_start(out=csl, in_=c_slab)
                ysl = consts.tile([BC, m_loc], F32)
                nc.scalar.activation(out=ysl, in_=csl, func=ACTF.Identity,
                                     bias=bia32, scale=scl32)

                # -- x0 local slab: transpose + pair-normalize (even/odd) --
                xsl = initp.tile([BC, m_loc], F32, tag="isl")
                nc.sync.dma_start(out=xsl, in_=x_slab)
                ps_x = psinit.tile([128, fw], F32, tag="ps_y")
                for mc in range(mch):
                    nc.tensor.transpose(ps_x[:, mc * BC:(mc + 1) * BC],
                                        xsl[:, mc * 128:(mc + 1) * 128], id32_sb)
                x0f = initp.tile([128, fw], F32)
                nc.vector.tensor_copy(x0f, ps_x)
                x0v = x0f.rearrange("p (mh eo h) -> p mh eo h", eo=2, h=HBC)
                xev = xe.rearrange("p (mh h) -> p mh h", h=HBC)
                xov = xo.rearrange("p (mh h) -> p mh h", h=HBC)
                t2 = initp.tile([128, hw], F32, tag="t2")
                u3 = initp.tile([128, hw], F32, tag="u3")
                nc.scalar.activation(out=t2.rearrange("p (mh h) -> p mh h", h=HBC),
                                     in_=x0v[:, :, 0], func=ACTF.Square)
                nc.scalar.activation(out=u3.rearrange("p (mh h) -> p mh h", h=HBC),
                                     in_=x0v[:, :, 1], func=ACTF.Square)
                ss = initp.tile([128, hw], F32, tag="ss")
                nc.vector.tensor_add(ss, t2, u3)
                nr = initp.tile([128, hw], F32, tag="nr")
                nc.scalar.activation(out=nr, in_=ss, func=ACTF.Sqrt,
                                     bias=eps12_sb, scale=1.0)
                rr = initp.tile([128, hw], F32, tag="rr")
                nc.vector.reciprocal_approx_fast(out=rr, in_=nr)
                nc.vector.tensor_mul(xev, x0v[:, :, 0], rr.rearrange(
                    "p (mh h) -> p mh h", h=HBC))
                nc.vector.tensor_mul(xov, x0v[:, :, 1], rr.rearrange(
                    "p (mh h) -> p mh h", h=HBC))
                # slab for AG#0
                sl0 = state.tile([128, fw], BF16, tag="sl")
                sl0v = sl0.rearrange("p (mh eo h) -> p mh eo h", eo=2, h=HBC)
                nc.vector.tensor_copy(sl0v[:, :, 0], xev)
                nc.vector.tensor_copy(sl0v[:, :, 1], xov)
                exchange(sl0, xcur_h)

            # ---------------- build A^T shard (+ step-0 matmuls) -----------
            # psa: 4-way column tiling, group j -> psum partitions 32j..32j+31,
            # bank j cols 0..255 (m-quarter j); streams run concurrently
            mq = m_loc // 4
            psa = psmm.tile([128, 4, 512], F32)
            piece = 2048
            with tc.tile_pool(name="bstage", bufs=3) as bstage, \
                 tc.tile_pool(name="bprod", bufs=1) as bprod, \
                 tc.tile_pool(name="pst", bufs=2, space="PSUM") as pst:
                for g in range(N_GROUPS):
                    atb = atbs[g]
                    atb_r = atb.rearrange("p (t m) -> p t m", m=m_loc)
                    for j in range(mch):
                        scp = bstage.tile([128, piece], F32, tag="scp")
                        nc.sync.dma_start(
                            out=scp,
                            in_=sc_s[j * 128:(j + 1) * 128,
                                     g * piece:(g + 1) * piece])
                        cwp = bstage.tile([128, piece], F32, tag="cwp")
                        nc.sync.dma_start(
                            out=cwp,
                            in_=cw_s[j * 128:(j + 1) * 128,
                                     g * piece:(g + 1) * piece])
                        prod = bprod.tile([128, piece], BF16, tag="prod")
                        nc.vector.tensor_mul(prod, scp, cwp)
                        for half in range(2):
                            pt = pst.tile([128, 8 * 128], BF16, tag="pt")
                            for tt in range(8):
                                t = half * 8 + tt
                                nc.tensor.transpose(
                                    pt[:, tt * 128:(tt + 1) * 128],
                                    prod[:, t * 128:(t + 1) * 128], id128_sb)
                            dst = atb_r[:, half * 8:(half + 1) * 8,
                                        j * 128:(j + 1) * 128]
                            nc.scalar.copy(
                                out=dst,
                                in_=pt.rearrange("p (t m) -> p t m", m=128))
                    # step-0 matmuls for this group's n-chunks
                    # (the constant y enters the accumulation as an extra
                    # K=32 identity chunk: psa += I.T @ y^T)
                    if g == 0:
                        for j in range(4):
                            nc.tensor.matmul(
                                psa[32 * j:32 * j + 32, j, 0:mq],
                                lhsT=id32_sb,
                                rhs=ysl[:, j * mq:(j + 1) * mq],
                                start=True, stop=False,
                                tile_position=(0, 32 * j))
                    for tl in range(gch):
                        t = g * gch + tl
                        for j in range(4):
                            nc.tensor.matmul(
                                psa[32 * j:32 * j + 32, j, 0:mq],
                                lhsT=xcur_h[t // 32][:, (t % 32) * BC:
                                                    (t % 32 + 1) * BC],
                                rhs=atb_r[:, tl, j * mq:(j + 1) * mq],
                                start=False, stop=(t == nch - 1),
                                tile_position=(0, 32 * j))

            # ---------------- Euler steps ----------------
            with tc.tile_pool(name="psf", bufs=2, space="PSUM") as psf, \
                 tc.tile_pool(name="ew", bufs=2) as ew:
                for k in range(q_steps):
                    if k > 0:
                        psa = psmm.tile([128, 4, 512], F32)
                        for j in range(4):
                            nc.tensor.matmul(
                                psa[32 * j:32 * j + 32, j, 0:mq],
                                lhsT=id32_sb,
                                rhs=ysl[:, j * mq:(j + 1) * mq],
                                start=True, stop=False,
                                tile_position=(0, 32 * j))
                        for t in range(nch):
                            atb_r = atbs[t // gch].rearrange(
                                "p (t m) -> p t m", m=m_loc)
                            for j in range(4):
                                nc.tensor.matmul(
                                    psa[32 * j:32 * j + 32, j, 0:mq],
                                    lhsT=xcur_h[t // 32][:, (t % 32) * BC:
                                                        (t % 32 + 1) * BC],
                                    rhs=atb_r[:, t % gch, j * mq:(j + 1) * mq],
                                    start=False, stop=(t == nch - 1),
                                    tile_position=(0, 32 * j))
                    # evict -> coupT [32, m_loc] f32, transpose -> [m, bc']
                    coupT = ew.tile([32, m_loc], F32, tag="coupT")
                    for j in range(4):
                        nc.vector.tensor_copy(coupT[:, j * mq:(j + 1) * mq],
                                              psa[32 * j:32 * j + 32, j, 0:mq])
                    psb = psf.tile([128, fw], F32)
                    for mc in range(mch):
                        nc.tensor.transpose(psb[:, mc * BC:(mc + 1) * BC],
                                            coupT[:, mc * 128:(mc + 1) * 128],
                                            id32_sb)
                    psbv = psb.rearrange("p (mh eo h) -> p mh eo h", eo=2, h=HBC)
                    pe_v = psbv[:, :, 0]
                    po_v = psbv[:, :, 1]
                    xev_ = xe.rearrange("p (mh h) -> p mh h", h=HBC)
                    xov_ = xo.rearrange("p (mh h) -> p mh h", h=HBC)
                    # update chain; psb already holds yt = coup + y
                    se = ew.tile([128, hw], F32, tag="se")
                    nc.vector.tensor_mul(
                        se.rearrange("p (mh h) -> p mh h", h=HBC), xev_, pe_v)
                    so = ew.tile([128, hw], F32, tag="so")
                    nc.vector.tensor_mul(
                        so.rearrange("p (mh h) -> p mh h", h=HBC), xov_, po_v)
                    sim = ew.tile([128, hw], F32, tag="sim")
                    nc.vector.tensor_add(sim, se, so)
                    # even half: xne = xe + g*(yt_e + w_e*xo - sim*xe)
                    u = ew.tile([128, hw], F32, tag="u")
                    nc.vector.tensor_mul(u, omg_sb[:, 0:hw], xo)
                    nc.vector.tensor_add(
                        u.rearrange("p (mh h) -> p mh h", h=HBC),
                        pe_v, u.rearrange("p (mh h) -> p mh h", h=HBC))
                    w = ew.tile([128, hw], F32, tag="w")
                    nc.vector.tensor_mul(w, sim, xe)
                    nc.vector.tensor_sub(u, u, w)
                    xne = ew.tile([128, hw], F32, tag="xne")
                    nc.vector.scalar_tensor_tensor(
                        out=xne, in0=u, scalar=gam_sb, in1=xe,
                        op0=ALU.mult, op1=ALU.add)
                    # odd half: xno = xo + g*(yt_o - w_o*xe - sim*xo)
                    v = ew.tile([128, hw], F32, tag="v")
                    nc.vector.tensor_mul(v, omg_sb[:, hw:2 * hw], xe)
                    nc.vector.tensor_add(
                        v.rearrange("p (mh h) -> p mh h", h=HBC),
                        po_v, v.rearrange("p (mh h) -> p mh h", h=HBC))
                    w2 = ew.tile([128, hw], F32, tag="w2")
                    nc.vector.tensor_mul(w2, sim, xo)
                    nc.vector.tensor_sub(v, v, w2)
                    xno = ew.tile([128, hw], F32, tag="xno")
                    nc.vector.scalar_tensor_tensor(
                        out=xno, in0=v, scalar=gam_sb, in1=xo,
                        op0=ALU.mult, op1=ALU.add)
                    # renormalize pairs
                    t2 = ew.tile([128, hw], F32, tag="t2")
                    nc.scalar.activation(out=t2, in_=xne, func=ACTF.Square)
                    u3 = ew.tile([128, hw], F32, tag="u3")
                    nc.scalar.activation(out=u3, in_=xno, func=ACTF.Square)
                    ss = ew.tile([128, hw], F32, tag="ss")
                    nc.vector.tensor_add(ss, t2, u3)
                    nr = ew.tile([128, hw], F32, tag="nr")
                    nc.scalar.activation(out=nr, in_=ss, func=ACTF.Sqrt,
                                         bias=eps12_sb, scale=1.0)
                    rr = ew.tile([128, hw], F32, tag="rr")
                    nc.vector.reciprocal_approx_fast(out=rr, in_=nr)
                    xe2 = state.tile([128, hw], F32, tag="xe")
                    nc.vector.tensor_mul(xe2, xne, rr)
                    xo2 = state.tile([128, hw], F32, tag="xo")
                    nc.vector.tensor_mul(xo2, xno, rr)
                    xe, xo = xe2, xo2
                    # PE-warming filler chain: occupies the tensor engine
                    # through the DVE + AllGather window so HAM keeps the PE
                    # at 2.4 GHz (tag-shared PSUM slot serializes the chain
                    # between this step's evictions and next step's matmuls)
                    if k < q_steps - 1:
                        fps = psmm.tile([128, 4, 512], F32, tag="psa")
                        n_fill = 88
                        for fi in range(n_fill):
                            nc.tensor.matmul(
                                fps[0:32, 0, 0:mh2],
                                lhsT=id128_sb[:, 0:32],
                                rhs=atbs[0].rearrange(
                                    "p (t m) -> p t m", m=m_loc)[:, 0, 0:mh2],
                                start=(fi == 0), stop=(fi == n_fill - 1),
                                tile_position=(0, 0))
                    # exchange slab (skip on last step)
                    if k < q_steps - 1:
                        sl = state.tile([128, fw], BF16, tag="sl")
                        slv = sl.rearrange("p (mh eo h) -> p mh eo h",
                                           eo=2, h=HBC)
                        nc.vector.tensor_copy(
                            slv[:, :, 0],
                            xe.rearrange("p (mh h) -> p mh h", h=HBC))
                        nc.vector.tensor_copy(
                            slv[:, :, 1],
                            xo.rearrange("p (mh h) -> p mh h", h=HBC))
                        xna = state.tile([128, nch * BC // 2], BF16,
                                         tag="xcur_a")
                        xnb = state.tile([128, nch * BC // 2], BF16,
                                         tag="xcur_b")
                        exchange(sl, [xna, xnb])
                        xcur_h = [xna, xnb]
                    # stash into the output accumulator (off critical path)
                    nc.scalar.copy(out=oa_r[:, k, :, 0],
                                   in_=xe.rearrange("p (mh h) -> p mh h", h=HBC))
                    nc.scalar.copy(out=oa_r[:, k, :, 1],
                                   in_=xo.rearrange("p (mh h) -> p mh h", h=HBC))

            nc.sync.dma_start(out=out_d, in_=outacc)

    nc.compile()
    nc.m = get_hw_module(nc.m)
    return nc


def _bc_perm():
    """Row order (eo, b, g) -> natural row index b*C + (2g+eo)."""
    perm = []
    for eo in range(2):
        for b in range(B):
            for g in range(C // 2):
                perm.append(b * C + 2 * g + eo)
    return np.array(perm, np.int64)


def make_inputs(x, c, sc, gn_w, gn_b, conn_w, omg_param, gamma,
                n=N_FULL, ncores=N_CORES):
    """Host-side marshalling: per-core input dicts."""
    m_loc = n // ncores
    bf16 = ml_dtypes.bfloat16
    perm = _bc_perm()

    x_nat = x.reshape(BC, n)
    c_nat = np.ascontiguousarray(c.reshape(BC, n), dtype=np.float32)
    gnw_i = np.ascontiguousarray(
        np.tile(gn_w.astype(np.float32), B)[perm][:, None])
    gnb_i = np.ascontiguousarray(
        np.tile(gn_b.astype(np.float32), B)[perm][:, None])

    omg = np.abs(omg_param.astype(np.float32)[:, 0])  # [C//2]
    half = np.tile(omg, B)          # [(b g)] = 16
    # [:hw] = +w tiled per (mh, b, g); [hw:] = -w
    pos = np.tile(half, m_loc // 128)
    omg_i = np.ascontiguousarray(
        np.concatenate([pos, -pos])[None, :].astype(np.float32))

    gam_i = np.asarray(gamma, np.float32).reshape(1, 1)

    sel2 = np.zeros((128, BC), np.float32)
    for p in range(128):
        a = p // 4  # channel-row (b, c) natural
        for j in range(BC):
            nat = perm[j]
            if a // 2 == nat // 2:
                sel2[p, j] = 1.0 / 8.0
    id32 = np.eye(32, dtype=np.float32)
    id128 = np.eye(128).astype(bf16)

    shared = dict(c_nat=c_nat, gnw_i=gnw_i, gnb_i=gnb_i,
                  omg_i=omg_i, gam_i=gam_i, sel2_i=sel2,
                  id32_i=id32, id128_i=id128)
    in_maps = []
    for r in range(ncores):
        sl = slice(r * m_loc, (r + 1) * m_loc)
        in_maps.append(dict(
            shared,
            sc_s=np.ascontiguousarray(sc[0, sl, :], dtype=np.float32),
            cw_s=np.ascontiguousarray(conn_w[sl, :], dtype=np.float32),
            x_slab=np.ascontiguousarray(x_nat[perm, sl]),
            c_slab=np.ascontiguousarray(c_nat[perm, sl]),
        ))
    return in_maps


def assemble_output(outs, n=N_FULL, ncores=N_CORES, q_steps=Q_STEPS):
    """outs: list of per-core out_d [128, q*fw] -> [Q, B, N, C] f32."""
    m_loc = n // ncores
    mch = m_loc // 128
    full = np.empty((q_steps, B, n, C), np.float32)
    for r in range(ncores):
        a = outs[r].reshape(128, q_steps, mch, 2, B, C // 2)
        # -> [k, b, mh, p, g, eo] -> m = mh*128+p, c = 2g+eo
        a = a.transpose(1, 4, 2, 0, 5, 3)
        full[:, :, r * m_loc:(r + 1) * m_loc, :] = a.reshape(
            q_steps, B, m_loc, C)
    return full


_PROGRAM_CACHE = {}


def get_program(n=N_FULL, ncores=N_CORES, q_steps=Q_STEPS):
    key = (n, ncores, q_steps)
    if key not in _PROGRAM_CACHE:
        _PROGRAM_CACHE[key] = build_program(n, ncores, q_steps)
    return _PROGRAM_CACHE[key]


def kernel(x, c, sc, gn_w, gn_b, conn_w, omg_param, gamma, Q):
    assert int(Q) == Q_STEPS
    x = np.asarray(x); c = np.asarray(c); sc = np.asarray(sc)
    gn_w = np.asarray(gn_w); gn_b = np.asarray(gn_b)
    conn_w = np.asarray(conn_w); omg_param = np.asarray(omg_param)
    gamma = np.asarray(gamma)
    n = x.shape[2]
    nc = get_program(n, N_CORES, Q_STEPS)
    in_maps = make_inputs(x, c, sc, gn_w, gn_b, conn_w, omg_param, gamma,
                          n=n, ncores=N_CORES)
    res = run_bass_kernel_spmd(nc, in_maps, core_ids=list(range(N_CORES)))
    outs = [res.results[r]["out_d"] for r in range(N_CORES)]
    return assemble_output(outs, n=n)
